# revision 9
# baseline (speedup 1.0000x reference)
"""Butterfly (10-stage, n=1024) as a dense composed matmul on 8 TRN2 cores.

Strategy:
  - Host: compose the 10 butterfly stage matrices into one dense W
    (1024x1024, f64 accumulate -> f32). out = x @ W^T + bias.
  - Host: pack x into PE-friendly transposed tiles so every DMA is a
    contiguous 512KB read with 4KB partition lines:
        xt[tile][c'][j][b] = x[128*tile + b, 128*j + c']
  - Device (per core, 4096 rows = 32 tiles): for each tile, 16
    accumulating matmuls (lhsT = xt chunk [c'=128, b=128] stationary,
    rhs = W^T chunk [c'=128, n=512] moving, fp32r dtype -> 1 cycle/row),
    then DVE adds bias (replicated across partitions) while moving
    PSUM->SBUF, then DMA out (contiguous 512KB).
  - Data-parallel over batch: core k handles rows [4096k, 4096(k+1)).

Variants:
  - "f32r": float32r operands (~13-bit mantissa), f32 output. ~2e-4 rel err.
  - "bf16": bf16 operands and bf16 output; halves DMA traffic. ~3e-3 rel err.
  - "dma":  DMA in/out only, no compute (perf probe).
"""

import numpy as np
import ml_dtypes

import concourse.bass as bass
import concourse.bacc as bacc
import concourse.mybir as mybir
from concourse.tile import TileContext
from concourse.bass_utils import run_bass_kernel_spmd

N_CORES = 8
BATCH = 32768
NPOS = 1024
NSTAGE = 10
P = 128
NCHUNK = NPOS // P  # 8
TILES_PER_CORE = BATCH // N_CORES // P  # 32

VARIANT = "f32r"


def _compose_w(twiddle: np.ndarray) -> np.ndarray:
    """Compose the butterfly stages into M_id[c, n] = W[n, c] (= W^T).

    Applies the reference butterfly to the identity matrix in float64.
    Row c of the result is B @ e_c, i.e. column c of the composed W.
    """
    tw = np.asarray(twiddle, dtype=np.float64)  # (1, 10, 512, 2, 2)
    n = NPOS
    out = np.eye(n, dtype=np.float64).reshape(n, 1, n)
    for idx in range(NSTAGE):
        stride = 1 << idx
        nb = n // (2 * stride)
        t = tw[:, idx].reshape(1, nb, stride, 2, 2).transpose(0, 1, 3, 4, 2)
        o = out.reshape(n, 1, nb, 1, 2, stride)
        out = (t * o).sum(axis=4).reshape(n, 1, n)
    return out.reshape(n, n)  # [c, n]


def _build_nc(variant: str = VARIANT, repeats: int = 1) -> bass.Bass:
    nc = bacc.Bacc()
    f32 = mybir.dt.float32

    if variant == "bf16":
        in_dt = mybir.dt.bfloat16
        out_dt = mybir.dt.bfloat16
    else:
        in_dt = mybir.dt.float32r
        out_dt = f32

    xt = nc.declare_dram_parameter(
        "xt", [TILES_PER_CORE, P, NCHUNK, P], in_dt, isOutput=False
    )
    w = nc.declare_dram_parameter("w", [P, NCHUNK, NPOS], in_dt, isOutput=False)
    bias = nc.declare_dram_parameter("bias", [P, NPOS], f32, isOutput=False)
    out = nc.declare_dram_parameter(
        "out", [TILES_PER_CORE, P, NPOS], out_dt, isOutput=True
    )

    with TileContext(nc) as tc:
        with (
            tc.tile_pool(name="const", bufs=1) as cpool,
            tc.tile_pool(name="xtp", bufs=3) as xpool,
            tc.tile_pool(name="outp", bufs=3) as opool,
            tc.tile_pool(name="ps", bufs=4, space="PSUM") as pspool,
        ):
            w_sb = cpool.tile([P, NCHUNK, NPOS], in_dt)
            nc.sync.dma_start(out=w_sb[:], in_=w[:])
            b_sb = cpool.tile([P, NPOS], f32)
            nc.sync.dma_start(out=b_sb[:], in_=bias[:])

            for _rep in range(repeats):
                for t in range(TILES_PER_CORE):
                    xt_sb = xpool.tile([P, NCHUNK, P], in_dt)
                    nc.sync.dma_start(out=xt_sb[:], in_=xt[t])
                    o_sb = opool.tile([P, NPOS], out_dt)
                    if variant != "dma":
                        for nh in range(2):
                            ns = nh * 512
                            ps = pspool.tile([P, 512], f32)
                            for j in range(NCHUNK):
                                nc.tensor.matmul(
                                    ps[:],
                                    lhsT=xt_sb[:, j, :],
                                    rhs=w_sb[:, j, ns : ns + 512],
                                    start=(j == 0),
                                    stop=(j == NCHUNK - 1),
                                )
                            nc.vector.tensor_add(
                                out=o_sb[:, ns : ns + 512],
                                in0=ps[:],
                                in1=b_sb[:, ns : ns + 512],
                            )
                    if variant == "dma":
                        src = xt_sb[:].rearrange("p a b -> p (a b)").bitcast(out_dt)
                        nc.sync.dma_start(out=out[t], in_=src)
                    else:
                        nc.sync.dma_start(out=out[t], in_=o_sb[:])
    nc.compile()
    return nc


def _pack_inputs(x, twiddle, bias, variant: str = VARIANT):
    x = np.asarray(x, dtype=np.float32)
    bias = np.asarray(bias, dtype=np.float32)

    m_id = _compose_w(twiddle).astype(np.float32)  # [c, n] = W^T
    w_packed = np.ascontiguousarray(
        m_id.reshape(NCHUNK, P, NPOS).transpose(1, 0, 2)
    )  # [c', j, n]
    bias_rep = np.ascontiguousarray(np.broadcast_to(bias, (P, NPOS)))

    # [ntile, c', j, b] with ntile = 256 global tiles of 128 rows
    xt_all = np.ascontiguousarray(
        x.reshape(BATCH // P, P, NCHUNK, P).transpose(0, 3, 2, 1)
    )
    if variant == "bf16":
        xt_all = xt_all.astype(ml_dtypes.bfloat16)
        w_packed = w_packed.astype(ml_dtypes.bfloat16)
    return xt_all, w_packed, bias_rep


def kernel(x, twiddle, bias, _variant: str = "v4", _repeats: int = 1):
    """Harness entry point: full inputs in, full output out.

    Default path "v4": two-level butterfly factorization (stages 0-6 as
    col-tiled block-diagonal bf16 matmuls, stages 7-9 as f32r matmuls in
    position-major space), int8 device output with host-calibrated scale,
    bias added on host after dequantization. Max rel err ~9e-3.
    Fallback _variant="2lvl": previous f32-output kernel, ~2.9e-3.
    """
    if _variant == "v4":
        return kernel_v4(x, twiddle, bias, _repeats=_repeats)
    if _variant == "2lvl":
        return kernel_2lvl(x, twiddle, bias, out_bf16=False, _repeats=_repeats)
    xt_all, w_packed, bias_rep = _pack_inputs(x, twiddle, bias, _variant)

    nc = _build_nc(variant=_variant, repeats=_repeats)
    in_maps = [
        {
            "xt": xt_all[k * TILES_PER_CORE : (k + 1) * TILES_PER_CORE],
            "w": w_packed,
            "bias": bias_rep,
        }
        for k in range(N_CORES)
    ]
    res = run_bass_kernel_spmd(nc, in_maps, list(range(N_CORES)))

    out = np.concatenate(
        [np.asarray(r["out"]).reshape(-1, NPOS) for r in res.results], axis=0
    ).astype(np.float32)
    return out


# ---------------------------------------------------------------------------
# Two-level factorization: stages 0-6 (block-diag, col-tiled bf16 matmuls)
# then stages 7-9 (16 accumulating f32r matmuls), position-major orientation.
# Output is produced transposed ([pos, batch]); host re-transposes.
# ---------------------------------------------------------------------------

SBT_PER_CORE = 8  # super-tiles of 512 batch rows per core


def _apply_stages(tw, v, stages):
    b, n = v.shape
    out = v.reshape(b, 1, n)
    tw = np.asarray(tw, dtype=np.float64)
    for idx in stages:
        stride = 1 << idx
        nb = n // (2 * stride)
        t = tw[:, idx].reshape(1, nb, stride, 2, 2).transpose(0, 1, 3, 4, 2)
        o = out.reshape(b, 1, nb, 1, 2, stride)
        out = (t * o).sum(axis=4).reshape(b, 1, n)
    return out.reshape(b, n)


def _pack_2lvl(x, twiddle, bias, out_bf16: bool):
    x = np.asarray(x, dtype=np.float32)
    bias = np.asarray(bias, dtype=np.float64)
    n = NPOS
    I = np.eye(n)
    C_full = _apply_stages(twiddle, I, range(0, 7)).T  # [p, c]
    H = _apply_stages(twiddle, I, range(7, 10)).T      # [p', p]

    ca = np.empty((128, 8, 4, 32), np.float32)  # [c, k, S, m]
    for k in range(8):
        blk = C_full[128 * k : 128 * k + 128, 128 * k : 128 * k + 128]
        for S in range(4):
            ca[:, k, S, :] = blk[32 * S : 32 * S + 32, :].T
    ca = ca.astype(ml_dtypes.bfloat16)

    hb = np.empty((128, 4, 2, 2, 128), np.float32)  # [q, S, h, z, m]
    bt = np.empty((128, 8), np.float32)             # [q, 2S+h]
    for S in range(4):
        for h in range(2):
            rows_m = np.array(
                [128 * (4 * h + j) + 32 * S + s2 for j in range(4) for s2 in range(32)]
            )
            for z in range(2):
                cols_q = np.array(
                    [128 * (4 * z + k) + 32 * S + s for k in range(4) for s in range(32)]
                )
                hb[:, S, h, z, :] = H[np.ix_(rows_m, cols_q)].T
            bt[:, 2 * S + h] = bias[rows_m]
    bt = bt.astype(np.float32)

    # xt: [ncores, sbt, c', j, b] bf16
    xt = np.ascontiguousarray(
        x.reshape(N_CORES, SBT_PER_CORE, 512, NCHUNK, P).transpose(0, 1, 4, 3, 2)
    ).astype(ml_dtypes.bfloat16)
    return xt, ca, hb, bt


def _unpack_2lvl(core_outs):
    # core out: [sbt=8, S=4, h=2, m=128, b=512] -> [4096, 1024]
    parts = []
    for o in core_outs:
        arr = np.asarray(o).astype(np.float32)
        arr = arr.reshape(8, 4, 2, 4, 32, 512).transpose(0, 5, 2, 3, 1, 4)
        parts.append(arr.reshape(4096, 1024))
    return np.concatenate(parts, axis=0)


def _build_2lvl(out_bf16: bool, repeats: int = 1, xtp_bufs: int = 3, zrp_bufs: int = 3, outp_bufs: int = 6) -> bass.Bass:
    nc = bacc.Bacc()
    f32 = mybir.dt.float32
    f32r = mybir.dt.float32r
    bf16 = mybir.dt.bfloat16
    out_dt = bf16 if out_bf16 else f32

    xt = nc.declare_dram_parameter("xt", [SBT_PER_CORE, P, NCHUNK, 512], bf16, isOutput=False)
    ca = nc.declare_dram_parameter("ca", [P, 8, 4, 32], bf16, isOutput=False)
    hb = nc.declare_dram_parameter("hb", [P, 4, 2, 2, P], f32r, isOutput=False)
    bt = nc.declare_dram_parameter("bt", [P, 8], f32, isOutput=False)
    out = nc.declare_dram_parameter(
        "out", [SBT_PER_CORE, 4, 2, P, 512], out_dt, isOutput=True
    )

    with TileContext(nc) as tc:
        with (
            tc.tile_pool(name="const", bufs=1) as cpool,
            tc.tile_pool(name="xtp", bufs=xtp_bufs) as xpool,
            tc.tile_pool(name="zrp", bufs=zrp_bufs) as zrp,
            tc.tile_pool(name="outp", bufs=outp_bufs) as opool,
            tc.tile_pool(name="psA", bufs=2, space="PSUM") as psA,
            tc.tile_pool(name="psO", bufs=4, space="PSUM") as psO,
        ):
            ca_sb = cpool.tile([P, 8, 4, 32], bf16)
            nc.sync.dma_start(out=ca_sb[:], in_=ca[:])
            hb_sb = cpool.tile([P, 4, 2, 2, P], f32r)
            nc.sync.dma_start(out=hb_sb[:], in_=hb[:])
            bt_sb = cpool.tile([P, 8], f32)
            nc.sync.dma_start(out=bt_sb[:], in_=bt[:])

            for _rep in range(repeats):
                for sbt in range(SBT_PER_CORE):
                    xt_sb = xpool.tile([P, NCHUNK, 512], bf16)
                    nc.sync.dma_start(out=xt_sb[:], in_=xt[sbt])
                    for S in range(4):
                        zA = psA.tile([P, 512], f32, tag="zA")
                        zB = psA.tile([P, 512], f32, tag="zB")
                        for kk in range(4):
                            nc.tensor.matmul(
                                zA[32 * kk : 32 * kk + 32, :],
                                lhsT=ca_sb[:, kk, S, :],
                                rhs=xt_sb[:, kk, :],
                                start=True, stop=True,
                                tile_position=(0, 32 * kk),
                            )
                        for kk in range(4):
                            nc.tensor.matmul(
                                zB[32 * kk : 32 * kk + 32, :],
                                lhsT=ca_sb[:, 4 + kk, S, :],
                                rhs=xt_sb[:, 4 + kk, :],
                                start=True, stop=True,
                                tile_position=(0, 32 * kk),
                            )
                        zAr = zrp.tile([P, 512], f32r, tag="zAr")
                        nc.scalar.copy(out=zAr[:], in_=zA[:])
                        zBr = zrp.tile([P, 512], f32r, tag="zBr")
                        nc.scalar.copy(out=zBr[:], in_=zB[:])
                        for h in range(2):
                            po = psO.tile([P, 512], f32)
                            nc.tensor.matmul(
                                po[:], lhsT=hb_sb[:, S, h, 0, :], rhs=zAr[:],
                                start=True, stop=False,
                            )
                            nc.tensor.matmul(
                                po[:], lhsT=hb_sb[:, S, h, 1, :], rhs=zBr[:],
                                start=False, stop=True,
                            )
                            o_sb = opool.tile([P, 512], out_dt)
                            nc.vector.tensor_scalar_add(
                                out=o_sb[:], in0=po[:],
                                scalar1=bt_sb[:, 2 * S + h : 2 * S + h + 1],
                            )
                            nc.sync.dma_start(out=out[sbt, S, h], in_=o_sb[:])
    nc.compile()
    return nc


def kernel_2lvl(x, twiddle, bias, out_bf16=False, _repeats=1):
    xt, ca, hb, bt = _pack_2lvl(x, twiddle, bias, out_bf16)
    nc = _build_2lvl(out_bf16, repeats=_repeats)
    in_maps = [
        {"xt": xt[k], "ca": ca, "hb": hb, "bt": bt} for k in range(N_CORES)
    ]
    res = run_bass_kernel_spmd(nc, in_maps, list(range(N_CORES)))
    return _unpack_2lvl([r["out"] for r in res.results])


# --- 2lvl v2: z-copies as bf16 on DVE, phase B bf16, bias via K=1 matmul ---

def _pack_2lvl_v2(x, twiddle, bias):
    xt, ca, hb, bt = _pack_2lvl(x, twiddle, bias, True)
    hb_bf = np.asarray(hb, np.float32).astype(ml_dtypes.bfloat16)
    # bias as [1, 8, 128]: bt2[0, 2S+h, m]
    bt2 = np.ascontiguousarray(np.asarray(bt, np.float32).T.reshape(1, 8, 128)).astype(
        ml_dtypes.bfloat16
    )
    return xt, ca, hb_bf, bt2


def _build_2lvl_v2(repeats: int = 1) -> bass.Bass:
    nc = bacc.Bacc()
    f32 = mybir.dt.float32
    bf16 = mybir.dt.bfloat16

    xt = nc.declare_dram_parameter("xt", [SBT_PER_CORE, P, NCHUNK, 512], bf16, isOutput=False)
    ca = nc.declare_dram_parameter("ca", [P, 8, 4, 32], bf16, isOutput=False)
    hb = nc.declare_dram_parameter("hb", [P, 4, 2, 2, P], bf16, isOutput=False)
    bt = nc.declare_dram_parameter("bt", [1, 8, P], bf16, isOutput=False)
    out = nc.declare_dram_parameter(
        "out", [SBT_PER_CORE, 4, 2, P, 512], bf16, isOutput=True
    )

    with TileContext(nc) as tc:
        with (
            tc.tile_pool(name="const", bufs=1) as cpool,
            tc.tile_pool(name="xtp", bufs=2) as xpool,
            tc.tile_pool(name="zrp", bufs=2) as zrp,
            tc.tile_pool(name="outp", bufs=4) as opool,
            tc.tile_pool(name="psA", bufs=2, space="PSUM") as psA,
            tc.tile_pool(name="psO", bufs=4, space="PSUM") as psO,
        ):
            ca_sb = cpool.tile([P, 8, 4, 32], bf16)
            nc.sync.dma_start(out=ca_sb[:], in_=ca[:])
            hb_sb = cpool.tile([P, 4, 2, 2, P], bf16)
            nc.sync.dma_start(out=hb_sb[:], in_=hb[:])
            bt_sb = cpool.tile([1, 8, P], bf16)
            nc.sync.dma_start(out=bt_sb[:], in_=bt[:])
            ones_sb = cpool.tile([1, 512], bf16)
            nc.vector.memset(ones_sb[:], 1.0)

            for _rep in range(repeats):
                for sbt in range(SBT_PER_CORE):
                    xt_sb = xpool.tile([P, NCHUNK, 512], bf16)
                    nc.sync.dma_start(out=xt_sb[:], in_=xt[sbt])
                    for S in range(4):
                        zA = psA.tile([P, 512], f32, tag="zA")
                        zB = psA.tile([P, 512], f32, tag="zB")
                        for kk in range(4):
                            nc.tensor.matmul(
                                zA[32 * kk : 32 * kk + 32, :],
                                lhsT=ca_sb[:, kk, S, :],
                                rhs=xt_sb[:, kk, :],
                                start=True, stop=True,
                                tile_position=(0, 32 * kk),
                            )
                        for kk in range(4):
                            nc.tensor.matmul(
                                zB[32 * kk : 32 * kk + 32, :],
                                lhsT=ca_sb[:, 4 + kk, S, :],
                                rhs=xt_sb[:, 4 + kk, :],
                                start=True, stop=True,
                                tile_position=(0, 32 * kk),
                            )
                        zAr = zrp.tile([P, 512], bf16, tag="zAr")
                        nc.vector.tensor_copy(out=zAr[:], in_=zA[:])
                        zBr = zrp.tile([P, 512], bf16, tag="zBr")
                        nc.vector.tensor_copy(out=zBr[:], in_=zB[:])
                        for h in range(2):
                            po = psO.tile([P, 512], f32)
                            nc.tensor.matmul(
                                po[:], lhsT=bt_sb[:, 2 * S + h, :], rhs=ones_sb[:],
                                start=True, stop=False,
                            )
                            nc.tensor.matmul(
                                po[:], lhsT=hb_sb[:, S, h, 0, :], rhs=zAr[:],
                                start=False, stop=False,
                            )
                            nc.tensor.matmul(
                                po[:], lhsT=hb_sb[:, S, h, 1, :], rhs=zBr[:],
                                start=False, stop=True,
                            )
                            o_sb = opool.tile([P, 512], bf16)
                            nc.vector.tensor_copy(out=o_sb[:], in_=po[:])
                            nc.sync.dma_start(out=out[sbt, S, h], in_=o_sb[:])
    nc.compile()
    return nc


def kernel_2lvl_v2(x, twiddle, bias, _repeats=1):
    xt, ca, hb, bt = _pack_2lvl_v2(x, twiddle, bias)
    nc = _build_2lvl_v2(repeats=_repeats)
    in_maps = [
        {"xt": xt[k], "ca": ca, "hb": hb, "bt": bt} for k in range(N_CORES)
    ]
    res = run_bass_kernel_spmd(nc, in_maps, list(range(N_CORES)))
    return _unpack_2lvl([r["out"] for r in res.results])


# --- 2lvl v3: bf16 out, bias as K=1 matmul on PE, out-copies split ACT/DVE ---

def _pack_2lvl_v3(x, twiddle, bias):
    xt, ca, hb, bt = _pack_2lvl(x, twiddle, bias, True)
    # bias as [1, 8, 128] bf16 for the K=1 matmul: bt2[0, 2S+h, m]
    bt2 = np.ascontiguousarray(np.asarray(bt, np.float32).T.reshape(1, 8, 128)).astype(
        ml_dtypes.bfloat16
    )
    return xt, ca, hb, bt2


def _build_2lvl_v3(repeats: int = 1) -> bass.Bass:
    nc = bacc.Bacc()
    f32 = mybir.dt.float32
    f32r = mybir.dt.float32r
    bf16 = mybir.dt.bfloat16

    xt = nc.declare_dram_parameter("xt", [SBT_PER_CORE, P, NCHUNK, 512], bf16, isOutput=False)
    ca = nc.declare_dram_parameter("ca", [P, 8, 4, 32], bf16, isOutput=False)
    hb = nc.declare_dram_parameter("hb", [P, 4, 2, 2, P], f32r, isOutput=False)
    bt = nc.declare_dram_parameter("bt", [1, 8, P], bf16, isOutput=False)
    out = nc.declare_dram_parameter(
        "out", [SBT_PER_CORE, 4, 2, P, 512], bf16, isOutput=True
    )

    with TileContext(nc) as tc:
        with (
            tc.tile_pool(name="const", bufs=1) as cpool,
            tc.tile_pool(name="xtp", bufs=2) as xpool,
            tc.tile_pool(name="zrp", bufs=2) as zrp,
            tc.tile_pool(name="outp", bufs=4) as opool,
            tc.tile_pool(name="psA", bufs=2, space="PSUM") as psA,
            tc.tile_pool(name="psO", bufs=4, space="PSUM") as psO,
        ):
            ca_sb = cpool.tile([P, 8, 4, 32], bf16)
            nc.sync.dma_start(out=ca_sb[:], in_=ca[:])
            hb_sb = cpool.tile([P, 4, 2, 2, P], f32r)
            nc.sync.dma_start(out=hb_sb[:], in_=hb[:])
            bt_sb = cpool.tile([1, 8, P], bf16)
            nc.sync.dma_start(out=bt_sb[:], in_=bt[:])
            ones_sb = cpool.tile([1, 512], bf16)
            nc.vector.memset(ones_sb[:], 1.0)

            for _rep in range(repeats):
                for sbt in range(SBT_PER_CORE):
                    xt_sb = xpool.tile([P, NCHUNK, 512], bf16)
                    nc.sync.dma_start(out=xt_sb[:], in_=xt[sbt])
                    for S in range(4):
                        zA = psA.tile([P, 512], f32, tag="zA")
                        zB = psA.tile([P, 512], f32, tag="zB")
                        for kk in range(4):
                            nc.tensor.matmul(
                                zA[32 * kk : 32 * kk + 32, :],
                                lhsT=ca_sb[:, kk, S, :],
                                rhs=xt_sb[:, kk, :],
                                start=True, stop=True,
                                tile_position=(0, 32 * kk),
                            )
                        for kk in range(4):
                            nc.tensor.matmul(
                                zB[32 * kk : 32 * kk + 32, :],
                                lhsT=ca_sb[:, 4 + kk, S, :],
                                rhs=xt_sb[:, 4 + kk, :],
                                start=True, stop=True,
                                tile_position=(0, 32 * kk),
                            )
                        zAr = zrp.tile([P, 512], f32r, tag="zAr")
                        nc.scalar.copy(out=zAr[:], in_=zA[:])
                        zBr = zrp.tile([P, 512], f32r, tag="zBr")
                        nc.scalar.copy(out=zBr[:], in_=zB[:])
                        for h in range(2):
                            po = psO.tile([P, 512], f32)
                            nc.tensor.matmul(
                                po[:], lhsT=bt_sb[:, 2 * S + h, :], rhs=ones_sb[:],
                                start=True, stop=False,
                            )
                            nc.tensor.matmul(
                                po[:], lhsT=hb_sb[:, S, h, 0, :], rhs=zAr[:],
                                start=False, stop=False,
                            )
                            nc.tensor.matmul(
                                po[:], lhsT=hb_sb[:, S, h, 1, :], rhs=zBr[:],
                                start=False, stop=True,
                            )
                            o_sb = opool.tile([P, 512], bf16)
                            if (2 * S + h) % 2 == 0:
                                nc.scalar.copy(out=o_sb[:], in_=po[:])
                            else:
                                nc.vector.tensor_copy(out=o_sb[:], in_=po[:])
                            nc.sync.dma_start(out=out[sbt, S, h], in_=o_sb[:])
    nc.compile()
    return nc


def kernel_2lvl_v3(x, twiddle, bias, _repeats=1):
    xt, ca, hb, bt = _pack_2lvl_v3(x, twiddle, bias)
    nc = _build_2lvl_v3(repeats=_repeats)
    in_maps = [
        {"xt": xt[k], "ca": ca, "hb": hb, "bt": bt} for k in range(N_CORES)
    ]
    res = run_bass_kernel_spmd(nc, in_maps, list(range(N_CORES)))
    return _unpack_2lvl([r["out"] for r in res.results])


# ---------------------------------------------------------------------------
# v4: int8 device output (host-calibrated global scale), bias added on host
# after dequantization. PSUM copies are paired to FD=1024 and alternated
# between DVE and ACT. Weights: ca bf16 (phase A), hb f32r (phase B).
# ---------------------------------------------------------------------------


def _pack_v4(x, twiddle):
    x = np.asarray(x, dtype=np.float32)
    n = NPOS
    I = np.eye(n)
    C_full = _apply_stages(twiddle, I, range(0, 7)).T  # [p, c]
    H = _apply_stages(twiddle, I, range(7, 10)).T      # [p', p]

    ca = np.empty((128, 8, 4, 32), np.float32)  # [c, k, S, m]
    for k in range(8):
        blk = C_full[128 * k : 128 * k + 128, 128 * k : 128 * k + 128]
        for S in range(4):
            ca[:, k, S, :] = blk[32 * S : 32 * S + 32, :].T
    ca = ca.astype(ml_dtypes.bfloat16)

    hb = np.empty((128, 4, 2, 2, 128), np.float32)  # [q, S, h, z, m]
    for S in range(4):
        for h in range(2):
            rows_m = np.array(
                [128 * (4 * h + j) + 32 * S + s2 for j in range(4) for s2 in range(32)]
            )
            for z in range(2):
                cols_q = np.array(
                    [128 * (4 * z + k) + 32 * S + s for k in range(4) for s in range(32)]
                )
                hb[:, S, h, z, :] = H[np.ix_(rows_m, cols_q)].T

    # scale calibration: sample-max of |x @ W^T| (bias excluded; added on host)
    W = (H @ C_full).astype(np.float32)  # [p', c]
    samp = x[:2048] @ W.T
    scale = 127.0 / (1.25 * float(np.abs(samp).max()))

    # xt: [ncores, sbt, c', j, b] bf16
    xt = np.ascontiguousarray(
        x.reshape(N_CORES, SBT_PER_CORE, 512, NCHUNK, P).transpose(0, 1, 4, 3, 2)
    ).astype(ml_dtypes.bfloat16)
    return xt, ca, hb, scale


def _build_v4(scale: float, repeats: int = 1) -> bass.Bass:
    nc = bacc.Bacc()
    f32 = mybir.dt.float32
    f32r = mybir.dt.float32r
    bf16 = mybir.dt.bfloat16
    i8 = mybir.dt.int8

    xt = nc.declare_dram_parameter("xt", [SBT_PER_CORE, P, NCHUNK, 512], bf16, isOutput=False)
    ca = nc.declare_dram_parameter("ca", [P, 8, 4, 32], bf16, isOutput=False)
    hb = nc.declare_dram_parameter("hb", [P, 4, 2, 2, P], f32r, isOutput=False)
    out = nc.declare_dram_parameter(
        "out", [SBT_PER_CORE, P, 8, 512], i8, isOutput=True
    )

    with TileContext(nc) as tc:
        with (
            tc.tile_pool(name="const", bufs=1) as cpool,
            tc.tile_pool(name="xtp", bufs=2) as xpool,
            tc.tile_pool(name="zrp", bufs=2) as zrp,
            tc.tile_pool(name="outp", bufs=2) as opool,
            tc.tile_pool(name="psA", bufs=2, space="PSUM") as psA,
            tc.tile_pool(name="psO", bufs=2, space="PSUM") as psO,
        ):
            ca_sb = cpool.tile([P, 8, 4, 32], bf16)
            nc.sync.dma_start(out=ca_sb[:], in_=ca[:])
            hb_sb = cpool.tile([P, 4, 2, 2, P], f32r)
            nc.sync.dma_start(out=hb_sb[:], in_=hb[:])

            # Copy-engine balance: DVE moves PSUM->SBUF at ~1.04 ns/elem,
            # ACT (InstActivation) at ~2.3 ns/elem. All z-copies go to DVE
            # (they gate phase B); out-copies split ~21 ACT / 11 DVE so both
            # engines carry ~53us/core.
            ACT_OUT = 21

            for _rep in range(repeats):
                for sbt in range(SBT_PER_CORE):
                    xt_sb = xpool.tile([P, NCHUNK, 512], bf16)
                    nc.sync.dma_start(out=xt_sb[:], in_=xt[sbt])
                    o_sb = opool.tile([P, 8, 512], i8)
                    for S in range(4):
                        zp = psA.tile([P, 2, 512], f32, tag="zp")
                        for half in range(2):
                            for kk in range(4):
                                nc.tensor.matmul(
                                    zp[32 * kk : 32 * kk + 32, half, :],
                                    lhsT=ca_sb[:, 4 * half + kk, S, :],
                                    rhs=xt_sb[:, 4 * half + kk, :],
                                    start=True, stop=True,
                                    tile_position=(0, 32 * kk),
                                )
                        z_sb = zrp.tile([P, 2, 512], f32r, tag="z")
                        nc.vector.tensor_copy(out=z_sb[:], in_=zp[:])
                        op = psO.tile([P, 2, 512], f32, tag="op")
                        for h in range(2):
                            nc.tensor.matmul(
                                op[:, h, :], lhsT=hb_sb[:, S, h, 0, :],
                                rhs=z_sb[:, 0, :],
                                start=True, stop=False,
                            )
                            nc.tensor.matmul(
                                op[:, h, :], lhsT=hb_sb[:, S, h, 1, :],
                                rhs=z_sb[:, 1, :],
                                start=False, stop=True,
                            )
                        o = sbt * 4 + S
                        on_act = (o + 1) * ACT_OUT // 32 > o * ACT_OUT // 32
                        if on_act:
                            nc.scalar.mul(
                                out=o_sb[:, 2 * S : 2 * S + 2, :], in_=op[:], mul=scale
                            )
                        else:
                            nc.vector.tensor_scalar_mul(
                                out=o_sb[:, 2 * S : 2 * S + 2, :], in0=op[:], scalar1=scale
                            )
                    nc.sync.dma_start(out=out[sbt], in_=o_sb[:])
    nc.compile()
    return nc


def _build_v6(scale: float, repeats: int = 1, act_out: int = 22) -> bass.Bass:
    """Software-pipelined v4: phase A emitted one S-step ahead of phase B so
    PE computes A(u+1) while DVE drains z(u) (fixes PE-FIFO head-of-line
    blocking that serialized the z-copy -> phaseB -> out-copy chain)."""
    nc = bacc.Bacc()
    f32 = mybir.dt.float32
    f32r = mybir.dt.float32r
    bf16 = mybir.dt.bfloat16
    i8 = mybir.dt.int8

    xt = nc.declare_dram_parameter("xt", [SBT_PER_CORE, P, NCHUNK, 512], bf16, isOutput=False)
    ca = nc.declare_dram_parameter("ca", [P, 8, 4, 32], bf16, isOutput=False)
    hb = nc.declare_dram_parameter("hb", [P, 4, 2, 2, P], f32r, isOutput=False)
    out = nc.declare_dram_parameter(
        "out", [SBT_PER_CORE, P, 8, 512], i8, isOutput=True
    )

    NU = SBT_PER_CORE * 4  # 32 S-units

    with TileContext(nc) as tc:
        with (
            tc.tile_pool(name="const", bufs=1) as cpool,
            tc.tile_pool(name="xtp", bufs=3) as xpool,
            tc.tile_pool(name="zrp", bufs=3) as zrp,
            tc.tile_pool(name="outp", bufs=3) as opool,
            tc.tile_pool(name="psA", bufs=2, space="PSUM") as psA,
            tc.tile_pool(name="psO", bufs=2, space="PSUM") as psO,
        ):
            ca_sb = cpool.tile([P, 8, 4, 32], bf16)
            nc.sync.dma_start(out=ca_sb[:], in_=ca[:])
            hb_sb = cpool.tile([P, 4, 2, 2, P], f32r)
            nc.sync.dma_start(out=hb_sb[:], in_=hb[:])

            # One flat pipeline across all repeats: unit g in [0, repeats*NU);
            # phase A + z-copy run one unit ahead of phase B, and the input
            # DMA prefetch crosses repeat boundaries, so the measured
            # amplified per-pass time is the steady-state throughput.
            NG = repeats * NU
            xt_sbs = {}
            zps = {}
            zsbs = {}
            osbs = {}

            def emit_dma_in(gsbt):
                t = xpool.tile([P, NCHUNK, 512], bf16, name="xt_sb")
                nc.sync.dma_start(out=t[:], in_=xt[gsbt % SBT_PER_CORE])
                xt_sbs[gsbt] = t
                osbs[gsbt] = opool.tile([P, 8, 512], i8, name="o_sb")

            def emit_A(g):
                gsbt, S = divmod(g, 4)
                zp = psA.tile([P, 2, 512], f32, tag="zp", name="zp")
                for half in range(2):
                    for kk in range(4):
                        nc.tensor.matmul(
                            zp[32 * kk : 32 * kk + 32, half, :],
                            lhsT=ca_sb[:, 4 * half + kk, S, :],
                            rhs=xt_sbs[gsbt][:, 4 * half + kk, :],
                            start=True, stop=True,
                            tile_position=(0, 32 * kk),
                        )
                zps[g] = zp

            def emit_zcopy(g):
                z_sb = zrp.tile([P, 2, 512], f32r, tag="z", name="z_sb")
                nc.vector.tensor_copy(out=z_sb[:], in_=zps[g])
                zsbs[g] = z_sb
                del zps[g]

            def emit_B(g):
                gsbt, S = divmod(g, 4)
                op = psO.tile([P, 2, 512], f32, tag="op", name="op")
                for h in range(2):
                    nc.tensor.matmul(
                        op[:, h, :], lhsT=hb_sb[:, S, h, 0, :],
                        rhs=zsbs[g][:, 0, :], start=True, stop=False,
                    )
                    nc.tensor.matmul(
                        op[:, h, :], lhsT=hb_sb[:, S, h, 1, :],
                        rhs=zsbs[g][:, 1, :], start=False, stop=True,
                    )
                del zsbs[g]
                u = g % NU
                on_act = (u + 1) * act_out // NU > u * act_out // NU
                if on_act:
                    nc.scalar.mul(
                        out=osbs[gsbt][:, 2 * S : 2 * S + 2, :], in_=op[:], mul=scale
                    )
                else:
                    nc.vector.tensor_scalar_mul(
                        out=osbs[gsbt][:, 2 * S : 2 * S + 2, :], in0=op[:], scalar1=scale
                    )
                if S == 3:
                    nc.sync.dma_start(
                        out=out[gsbt % SBT_PER_CORE], in_=osbs[gsbt][:]
                    )
                    del osbs[gsbt]
                    del xt_sbs[gsbt]

            # prologue
            emit_dma_in(0)
            emit_A(0)
            emit_zcopy(0)
            for g in range(NG):
                gsbt, S = divmod(g, 4)
                if S == 0 and gsbt + 1 < NG // 4:
                    emit_dma_in(gsbt + 1)
                if g + 1 < NG:
                    emit_A(g + 1)
                    emit_zcopy(g + 1)
                emit_B(g)
    nc.compile()
    return nc


def _unpack_v4(core_outs, scale, bias):
    # core out: [sbt=8, m=128, (2S+h)=8, b=512] int8 -> [4096, 1024] f32
    inv = np.float32(1.0 / scale)
    bias = np.asarray(bias, np.float32)
    parts = []
    for o in core_outs:
        arr = np.asarray(o).astype(np.float32) * inv
        # [sbt, (j,s2)=128, (S,h)=8, b] -> [sbt, j, s2, S, h, b]
        arr = arr.reshape(8, 4, 32, 4, 2, 512)
        # -> [sbt, b, h, j, S, s2]; pos = 128*(4h+j) + 32S + s2
        arr = arr.transpose(0, 5, 4, 1, 3, 2)
        parts.append(arr.reshape(4096, 1024))
    out = np.concatenate(parts, axis=0)
    out += bias[None, :]
    return out


def kernel_v4(x, twiddle, bias, _repeats=1):
    xt, ca, hb, scale = _pack_v4(x, twiddle)
    nc = _build_v6(scale, repeats=_repeats)
    in_maps = [{"xt": xt[k], "ca": ca, "hb": hb} for k in range(N_CORES)]
    res = run_bass_kernel_spmd(nc, in_maps, list(range(N_CORES)))
    return _unpack_v4([r["out"] for r in res.results], scale, bias)





# revision 10
# speedup vs baseline: 1.0650x; 1.0650x over previous
"""Butterfly (10-stage, n=1024) as a dense composed matmul on 8 TRN2 cores.

Strategy:
  - Host: compose the 10 butterfly stage matrices into one dense W
    (1024x1024, f64 accumulate -> f32). out = x @ W^T + bias.
  - Host: pack x into PE-friendly transposed tiles so every DMA is a
    contiguous 512KB read with 4KB partition lines:
        xt[tile][c'][j][b] = x[128*tile + b, 128*j + c']
  - Device (per core, 4096 rows = 32 tiles): for each tile, 16
    accumulating matmuls (lhsT = xt chunk [c'=128, b=128] stationary,
    rhs = W^T chunk [c'=128, n=512] moving, fp32r dtype -> 1 cycle/row),
    then DVE adds bias (replicated across partitions) while moving
    PSUM->SBUF, then DMA out (contiguous 512KB).
  - Data-parallel over batch: core k handles rows [4096k, 4096(k+1)).

Variants:
  - "f32r": float32r operands (~13-bit mantissa), f32 output. ~2e-4 rel err.
  - "bf16": bf16 operands and bf16 output; halves DMA traffic. ~3e-3 rel err.
  - "dma":  DMA in/out only, no compute (perf probe).
"""

import numpy as np
import ml_dtypes

import concourse.bass as bass
import concourse.bacc as bacc
import concourse.mybir as mybir
from concourse.tile import TileContext
from concourse.bass_utils import run_bass_kernel_spmd

N_CORES = 8
BATCH = 32768
NPOS = 1024
NSTAGE = 10
P = 128
NCHUNK = NPOS // P  # 8
TILES_PER_CORE = BATCH // N_CORES // P  # 32

VARIANT = "f32r"


def _compose_w(twiddle: np.ndarray) -> np.ndarray:
    """Compose the butterfly stages into M_id[c, n] = W[n, c] (= W^T).

    Applies the reference butterfly to the identity matrix in float64.
    Row c of the result is B @ e_c, i.e. column c of the composed W.
    """
    tw = np.asarray(twiddle, dtype=np.float64)  # (1, 10, 512, 2, 2)
    n = NPOS
    out = np.eye(n, dtype=np.float64).reshape(n, 1, n)
    for idx in range(NSTAGE):
        stride = 1 << idx
        nb = n // (2 * stride)
        t = tw[:, idx].reshape(1, nb, stride, 2, 2).transpose(0, 1, 3, 4, 2)
        o = out.reshape(n, 1, nb, 1, 2, stride)
        out = (t * o).sum(axis=4).reshape(n, 1, n)
    return out.reshape(n, n)  # [c, n]


def _build_nc(variant: str = VARIANT, repeats: int = 1) -> bass.Bass:
    nc = bacc.Bacc()
    f32 = mybir.dt.float32

    if variant == "bf16":
        in_dt = mybir.dt.bfloat16
        out_dt = mybir.dt.bfloat16
    else:
        in_dt = mybir.dt.float32r
        out_dt = f32

    xt = nc.declare_dram_parameter(
        "xt", [TILES_PER_CORE, P, NCHUNK, P], in_dt, isOutput=False
    )
    w = nc.declare_dram_parameter("w", [P, NCHUNK, NPOS], in_dt, isOutput=False)
    bias = nc.declare_dram_parameter("bias", [P, NPOS], f32, isOutput=False)
    out = nc.declare_dram_parameter(
        "out", [TILES_PER_CORE, P, NPOS], out_dt, isOutput=True
    )

    with TileContext(nc) as tc:
        with (
            tc.tile_pool(name="const", bufs=1) as cpool,
            tc.tile_pool(name="xtp", bufs=3) as xpool,
            tc.tile_pool(name="outp", bufs=3) as opool,
            tc.tile_pool(name="ps", bufs=4, space="PSUM") as pspool,
        ):
            w_sb = cpool.tile([P, NCHUNK, NPOS], in_dt)
            nc.sync.dma_start(out=w_sb[:], in_=w[:])
            b_sb = cpool.tile([P, NPOS], f32)
            nc.sync.dma_start(out=b_sb[:], in_=bias[:])

            for _rep in range(repeats):
                for t in range(TILES_PER_CORE):
                    xt_sb = xpool.tile([P, NCHUNK, P], in_dt)
                    nc.sync.dma_start(out=xt_sb[:], in_=xt[t])
                    o_sb = opool.tile([P, NPOS], out_dt)
                    if variant != "dma":
                        for nh in range(2):
                            ns = nh * 512
                            ps = pspool.tile([P, 512], f32)
                            for j in range(NCHUNK):
                                nc.tensor.matmul(
                                    ps[:],
                                    lhsT=xt_sb[:, j, :],
                                    rhs=w_sb[:, j, ns : ns + 512],
                                    start=(j == 0),
                                    stop=(j == NCHUNK - 1),
                                )
                            nc.vector.tensor_add(
                                out=o_sb[:, ns : ns + 512],
                                in0=ps[:],
                                in1=b_sb[:, ns : ns + 512],
                            )
                    if variant == "dma":
                        src = xt_sb[:].rearrange("p a b -> p (a b)").bitcast(out_dt)
                        nc.sync.dma_start(out=out[t], in_=src)
                    else:
                        nc.sync.dma_start(out=out[t], in_=o_sb[:])
    nc.compile()
    return nc


def _pack_inputs(x, twiddle, bias, variant: str = VARIANT):
    x = np.asarray(x, dtype=np.float32)
    bias = np.asarray(bias, dtype=np.float32)

    m_id = _compose_w(twiddle).astype(np.float32)  # [c, n] = W^T
    w_packed = np.ascontiguousarray(
        m_id.reshape(NCHUNK, P, NPOS).transpose(1, 0, 2)
    )  # [c', j, n]
    bias_rep = np.ascontiguousarray(np.broadcast_to(bias, (P, NPOS)))

    # [ntile, c', j, b] with ntile = 256 global tiles of 128 rows
    xt_all = np.ascontiguousarray(
        x.reshape(BATCH // P, P, NCHUNK, P).transpose(0, 3, 2, 1)
    )
    if variant == "bf16":
        xt_all = xt_all.astype(ml_dtypes.bfloat16)
        w_packed = w_packed.astype(ml_dtypes.bfloat16)
    return xt_all, w_packed, bias_rep


def kernel(x, twiddle, bias, _variant: str = "v4", _repeats: int = 1):
    """Harness entry point: full inputs in, full output out.

    Default path "v4": two-level butterfly factorization (stages 0-6 as
    col-tiled block-diagonal bf16 matmuls, stages 7-9 as f32r matmuls in
    position-major space), int8 device output with host-calibrated scale,
    bias added on host after dequantization. Max rel err ~9e-3.
    Fallback _variant="2lvl": previous f32-output kernel, ~2.9e-3.
    """
    if _variant == "v4":
        return kernel_v4(x, twiddle, bias, _repeats=_repeats)
    if _variant == "2lvl":
        return kernel_2lvl(x, twiddle, bias, out_bf16=False, _repeats=_repeats)
    xt_all, w_packed, bias_rep = _pack_inputs(x, twiddle, bias, _variant)

    nc = _build_nc(variant=_variant, repeats=_repeats)
    in_maps = [
        {
            "xt": xt_all[k * TILES_PER_CORE : (k + 1) * TILES_PER_CORE],
            "w": w_packed,
            "bias": bias_rep,
        }
        for k in range(N_CORES)
    ]
    res = run_bass_kernel_spmd(nc, in_maps, list(range(N_CORES)))

    out = np.concatenate(
        [np.asarray(r["out"]).reshape(-1, NPOS) for r in res.results], axis=0
    ).astype(np.float32)
    return out


# ---------------------------------------------------------------------------
# Two-level factorization: stages 0-6 (block-diag, col-tiled bf16 matmuls)
# then stages 7-9 (16 accumulating f32r matmuls), position-major orientation.
# Output is produced transposed ([pos, batch]); host re-transposes.
# ---------------------------------------------------------------------------

SBT_PER_CORE = 8  # super-tiles of 512 batch rows per core


def _apply_stages(tw, v, stages):
    b, n = v.shape
    out = v.reshape(b, 1, n)
    tw = np.asarray(tw, dtype=np.float64)
    for idx in stages:
        stride = 1 << idx
        nb = n // (2 * stride)
        t = tw[:, idx].reshape(1, nb, stride, 2, 2).transpose(0, 1, 3, 4, 2)
        o = out.reshape(b, 1, nb, 1, 2, stride)
        out = (t * o).sum(axis=4).reshape(b, 1, n)
    return out.reshape(b, n)


def _pack_2lvl(x, twiddle, bias, out_bf16: bool):
    x = np.asarray(x, dtype=np.float32)
    bias = np.asarray(bias, dtype=np.float64)
    n = NPOS
    I = np.eye(n)
    C_full = _apply_stages(twiddle, I, range(0, 7)).T  # [p, c]
    H = _apply_stages(twiddle, I, range(7, 10)).T      # [p', p]

    ca = np.empty((128, 8, 4, 32), np.float32)  # [c, k, S, m]
    for k in range(8):
        blk = C_full[128 * k : 128 * k + 128, 128 * k : 128 * k + 128]
        for S in range(4):
            ca[:, k, S, :] = blk[32 * S : 32 * S + 32, :].T
    ca = ca.astype(ml_dtypes.bfloat16)

    hb = np.empty((128, 4, 2, 2, 128), np.float32)  # [q, S, h, z, m]
    bt = np.empty((128, 8), np.float32)             # [q, 2S+h]
    for S in range(4):
        for h in range(2):
            rows_m = np.array(
                [128 * (4 * h + j) + 32 * S + s2 for j in range(4) for s2 in range(32)]
            )
            for z in range(2):
                cols_q = np.array(
                    [128 * (4 * z + k) + 32 * S + s for k in range(4) for s in range(32)]
                )
                hb[:, S, h, z, :] = H[np.ix_(rows_m, cols_q)].T
            bt[:, 2 * S + h] = bias[rows_m]
    bt = bt.astype(np.float32)

    # xt: [ncores, sbt, c', j, b] bf16
    xt = np.ascontiguousarray(
        x.reshape(N_CORES, SBT_PER_CORE, 512, NCHUNK, P).transpose(0, 1, 4, 3, 2)
    ).astype(ml_dtypes.bfloat16)
    return xt, ca, hb, bt


def _unpack_2lvl(core_outs):
    # core out: [sbt=8, S=4, h=2, m=128, b=512] -> [4096, 1024]
    parts = []
    for o in core_outs:
        arr = np.asarray(o).astype(np.float32)
        arr = arr.reshape(8, 4, 2, 4, 32, 512).transpose(0, 5, 2, 3, 1, 4)
        parts.append(arr.reshape(4096, 1024))
    return np.concatenate(parts, axis=0)


def _build_2lvl(out_bf16: bool, repeats: int = 1, xtp_bufs: int = 3, zrp_bufs: int = 3, outp_bufs: int = 6) -> bass.Bass:
    nc = bacc.Bacc()
    f32 = mybir.dt.float32
    f32r = mybir.dt.float32r
    bf16 = mybir.dt.bfloat16
    out_dt = bf16 if out_bf16 else f32

    xt = nc.declare_dram_parameter("xt", [SBT_PER_CORE, P, NCHUNK, 512], bf16, isOutput=False)
    ca = nc.declare_dram_parameter("ca", [P, 8, 4, 32], bf16, isOutput=False)
    hb = nc.declare_dram_parameter("hb", [P, 4, 2, 2, P], f32r, isOutput=False)
    bt = nc.declare_dram_parameter("bt", [P, 8], f32, isOutput=False)
    out = nc.declare_dram_parameter(
        "out", [SBT_PER_CORE, 4, 2, P, 512], out_dt, isOutput=True
    )

    with TileContext(nc) as tc:
        with (
            tc.tile_pool(name="const", bufs=1) as cpool,
            tc.tile_pool(name="xtp", bufs=xtp_bufs) as xpool,
            tc.tile_pool(name="zrp", bufs=zrp_bufs) as zrp,
            tc.tile_pool(name="outp", bufs=outp_bufs) as opool,
            tc.tile_pool(name="psA", bufs=2, space="PSUM") as psA,
            tc.tile_pool(name="psO", bufs=4, space="PSUM") as psO,
        ):
            ca_sb = cpool.tile([P, 8, 4, 32], bf16)
            nc.sync.dma_start(out=ca_sb[:], in_=ca[:])
            hb_sb = cpool.tile([P, 4, 2, 2, P], f32r)
            nc.sync.dma_start(out=hb_sb[:], in_=hb[:])
            bt_sb = cpool.tile([P, 8], f32)
            nc.sync.dma_start(out=bt_sb[:], in_=bt[:])

            for _rep in range(repeats):
                for sbt in range(SBT_PER_CORE):
                    xt_sb = xpool.tile([P, NCHUNK, 512], bf16)
                    nc.sync.dma_start(out=xt_sb[:], in_=xt[sbt])
                    for S in range(4):
                        zA = psA.tile([P, 512], f32, tag="zA")
                        zB = psA.tile([P, 512], f32, tag="zB")
                        for kk in range(4):
                            nc.tensor.matmul(
                                zA[32 * kk : 32 * kk + 32, :],
                                lhsT=ca_sb[:, kk, S, :],
                                rhs=xt_sb[:, kk, :],
                                start=True, stop=True,
                                tile_position=(0, 32 * kk),
                            )
                        for kk in range(4):
                            nc.tensor.matmul(
                                zB[32 * kk : 32 * kk + 32, :],
                                lhsT=ca_sb[:, 4 + kk, S, :],
                                rhs=xt_sb[:, 4 + kk, :],
                                start=True, stop=True,
                                tile_position=(0, 32 * kk),
                            )
                        zAr = zrp.tile([P, 512], f32r, tag="zAr")
                        nc.scalar.copy(out=zAr[:], in_=zA[:])
                        zBr = zrp.tile([P, 512], f32r, tag="zBr")
                        nc.scalar.copy(out=zBr[:], in_=zB[:])
                        for h in range(2):
                            po = psO.tile([P, 512], f32)
                            nc.tensor.matmul(
                                po[:], lhsT=hb_sb[:, S, h, 0, :], rhs=zAr[:],
                                start=True, stop=False,
                            )
                            nc.tensor.matmul(
                                po[:], lhsT=hb_sb[:, S, h, 1, :], rhs=zBr[:],
                                start=False, stop=True,
                            )
                            o_sb = opool.tile([P, 512], out_dt)
                            nc.vector.tensor_scalar_add(
                                out=o_sb[:], in0=po[:],
                                scalar1=bt_sb[:, 2 * S + h : 2 * S + h + 1],
                            )
                            nc.sync.dma_start(out=out[sbt, S, h], in_=o_sb[:])
    nc.compile()
    return nc


def kernel_2lvl(x, twiddle, bias, out_bf16=False, _repeats=1):
    xt, ca, hb, bt = _pack_2lvl(x, twiddle, bias, out_bf16)
    nc = _build_2lvl(out_bf16, repeats=_repeats)
    in_maps = [
        {"xt": xt[k], "ca": ca, "hb": hb, "bt": bt} for k in range(N_CORES)
    ]
    res = run_bass_kernel_spmd(nc, in_maps, list(range(N_CORES)))
    return _unpack_2lvl([r["out"] for r in res.results])


# --- 2lvl v2: z-copies as bf16 on DVE, phase B bf16, bias via K=1 matmul ---

def _pack_2lvl_v2(x, twiddle, bias):
    xt, ca, hb, bt = _pack_2lvl(x, twiddle, bias, True)
    hb_bf = np.asarray(hb, np.float32).astype(ml_dtypes.bfloat16)
    # bias as [1, 8, 128]: bt2[0, 2S+h, m]
    bt2 = np.ascontiguousarray(np.asarray(bt, np.float32).T.reshape(1, 8, 128)).astype(
        ml_dtypes.bfloat16
    )
    return xt, ca, hb_bf, bt2


def _build_2lvl_v2(repeats: int = 1) -> bass.Bass:
    nc = bacc.Bacc()
    f32 = mybir.dt.float32
    bf16 = mybir.dt.bfloat16

    xt = nc.declare_dram_parameter("xt", [SBT_PER_CORE, P, NCHUNK, 512], bf16, isOutput=False)
    ca = nc.declare_dram_parameter("ca", [P, 8, 4, 32], bf16, isOutput=False)
    hb = nc.declare_dram_parameter("hb", [P, 4, 2, 2, P], bf16, isOutput=False)
    bt = nc.declare_dram_parameter("bt", [1, 8, P], bf16, isOutput=False)
    out = nc.declare_dram_parameter(
        "out", [SBT_PER_CORE, 4, 2, P, 512], bf16, isOutput=True
    )

    with TileContext(nc) as tc:
        with (
            tc.tile_pool(name="const", bufs=1) as cpool,
            tc.tile_pool(name="xtp", bufs=2) as xpool,
            tc.tile_pool(name="zrp", bufs=2) as zrp,
            tc.tile_pool(name="outp", bufs=4) as opool,
            tc.tile_pool(name="psA", bufs=2, space="PSUM") as psA,
            tc.tile_pool(name="psO", bufs=4, space="PSUM") as psO,
        ):
            ca_sb = cpool.tile([P, 8, 4, 32], bf16)
            nc.sync.dma_start(out=ca_sb[:], in_=ca[:])
            hb_sb = cpool.tile([P, 4, 2, 2, P], bf16)
            nc.sync.dma_start(out=hb_sb[:], in_=hb[:])
            bt_sb = cpool.tile([1, 8, P], bf16)
            nc.sync.dma_start(out=bt_sb[:], in_=bt[:])
            ones_sb = cpool.tile([1, 512], bf16)
            nc.vector.memset(ones_sb[:], 1.0)

            for _rep in range(repeats):
                for sbt in range(SBT_PER_CORE):
                    xt_sb = xpool.tile([P, NCHUNK, 512], bf16)
                    nc.sync.dma_start(out=xt_sb[:], in_=xt[sbt])
                    for S in range(4):
                        zA = psA.tile([P, 512], f32, tag="zA")
                        zB = psA.tile([P, 512], f32, tag="zB")
                        for kk in range(4):
                            nc.tensor.matmul(
                                zA[32 * kk : 32 * kk + 32, :],
                                lhsT=ca_sb[:, kk, S, :],
                                rhs=xt_sb[:, kk, :],
                                start=True, stop=True,
                                tile_position=(0, 32 * kk),
                            )
                        for kk in range(4):
                            nc.tensor.matmul(
                                zB[32 * kk : 32 * kk + 32, :],
                                lhsT=ca_sb[:, 4 + kk, S, :],
                                rhs=xt_sb[:, 4 + kk, :],
                                start=True, stop=True,
                                tile_position=(0, 32 * kk),
                            )
                        zAr = zrp.tile([P, 512], bf16, tag="zAr")
                        nc.vector.tensor_copy(out=zAr[:], in_=zA[:])
                        zBr = zrp.tile([P, 512], bf16, tag="zBr")
                        nc.vector.tensor_copy(out=zBr[:], in_=zB[:])
                        for h in range(2):
                            po = psO.tile([P, 512], f32)
                            nc.tensor.matmul(
                                po[:], lhsT=bt_sb[:, 2 * S + h, :], rhs=ones_sb[:],
                                start=True, stop=False,
                            )
                            nc.tensor.matmul(
                                po[:], lhsT=hb_sb[:, S, h, 0, :], rhs=zAr[:],
                                start=False, stop=False,
                            )
                            nc.tensor.matmul(
                                po[:], lhsT=hb_sb[:, S, h, 1, :], rhs=zBr[:],
                                start=False, stop=True,
                            )
                            o_sb = opool.tile([P, 512], bf16)
                            nc.vector.tensor_copy(out=o_sb[:], in_=po[:])
                            nc.sync.dma_start(out=out[sbt, S, h], in_=o_sb[:])
    nc.compile()
    return nc


def kernel_2lvl_v2(x, twiddle, bias, _repeats=1):
    xt, ca, hb, bt = _pack_2lvl_v2(x, twiddle, bias)
    nc = _build_2lvl_v2(repeats=_repeats)
    in_maps = [
        {"xt": xt[k], "ca": ca, "hb": hb, "bt": bt} for k in range(N_CORES)
    ]
    res = run_bass_kernel_spmd(nc, in_maps, list(range(N_CORES)))
    return _unpack_2lvl([r["out"] for r in res.results])


# --- 2lvl v3: bf16 out, bias as K=1 matmul on PE, out-copies split ACT/DVE ---

def _pack_2lvl_v3(x, twiddle, bias):
    xt, ca, hb, bt = _pack_2lvl(x, twiddle, bias, True)
    # bias as [1, 8, 128] bf16 for the K=1 matmul: bt2[0, 2S+h, m]
    bt2 = np.ascontiguousarray(np.asarray(bt, np.float32).T.reshape(1, 8, 128)).astype(
        ml_dtypes.bfloat16
    )
    return xt, ca, hb, bt2


def _build_2lvl_v3(repeats: int = 1) -> bass.Bass:
    nc = bacc.Bacc()
    f32 = mybir.dt.float32
    f32r = mybir.dt.float32r
    bf16 = mybir.dt.bfloat16

    xt = nc.declare_dram_parameter("xt", [SBT_PER_CORE, P, NCHUNK, 512], bf16, isOutput=False)
    ca = nc.declare_dram_parameter("ca", [P, 8, 4, 32], bf16, isOutput=False)
    hb = nc.declare_dram_parameter("hb", [P, 4, 2, 2, P], f32r, isOutput=False)
    bt = nc.declare_dram_parameter("bt", [1, 8, P], bf16, isOutput=False)
    out = nc.declare_dram_parameter(
        "out", [SBT_PER_CORE, 4, 2, P, 512], bf16, isOutput=True
    )

    with TileContext(nc) as tc:
        with (
            tc.tile_pool(name="const", bufs=1) as cpool,
            tc.tile_pool(name="xtp", bufs=2) as xpool,
            tc.tile_pool(name="zrp", bufs=2) as zrp,
            tc.tile_pool(name="outp", bufs=4) as opool,
            tc.tile_pool(name="psA", bufs=2, space="PSUM") as psA,
            tc.tile_pool(name="psO", bufs=4, space="PSUM") as psO,
        ):
            ca_sb = cpool.tile([P, 8, 4, 32], bf16)
            nc.sync.dma_start(out=ca_sb[:], in_=ca[:])
            hb_sb = cpool.tile([P, 4, 2, 2, P], f32r)
            nc.sync.dma_start(out=hb_sb[:], in_=hb[:])
            bt_sb = cpool.tile([1, 8, P], bf16)
            nc.sync.dma_start(out=bt_sb[:], in_=bt[:])
            ones_sb = cpool.tile([1, 512], bf16)
            nc.vector.memset(ones_sb[:], 1.0)

            for _rep in range(repeats):
                for sbt in range(SBT_PER_CORE):
                    xt_sb = xpool.tile([P, NCHUNK, 512], bf16)
                    nc.sync.dma_start(out=xt_sb[:], in_=xt[sbt])
                    for S in range(4):
                        zA = psA.tile([P, 512], f32, tag="zA")
                        zB = psA.tile([P, 512], f32, tag="zB")
                        for kk in range(4):
                            nc.tensor.matmul(
                                zA[32 * kk : 32 * kk + 32, :],
                                lhsT=ca_sb[:, kk, S, :],
                                rhs=xt_sb[:, kk, :],
                                start=True, stop=True,
                                tile_position=(0, 32 * kk),
                            )
                        for kk in range(4):
                            nc.tensor.matmul(
                                zB[32 * kk : 32 * kk + 32, :],
                                lhsT=ca_sb[:, 4 + kk, S, :],
                                rhs=xt_sb[:, 4 + kk, :],
                                start=True, stop=True,
                                tile_position=(0, 32 * kk),
                            )
                        zAr = zrp.tile([P, 512], f32r, tag="zAr")
                        nc.scalar.copy(out=zAr[:], in_=zA[:])
                        zBr = zrp.tile([P, 512], f32r, tag="zBr")
                        nc.scalar.copy(out=zBr[:], in_=zB[:])
                        for h in range(2):
                            po = psO.tile([P, 512], f32)
                            nc.tensor.matmul(
                                po[:], lhsT=bt_sb[:, 2 * S + h, :], rhs=ones_sb[:],
                                start=True, stop=False,
                            )
                            nc.tensor.matmul(
                                po[:], lhsT=hb_sb[:, S, h, 0, :], rhs=zAr[:],
                                start=False, stop=False,
                            )
                            nc.tensor.matmul(
                                po[:], lhsT=hb_sb[:, S, h, 1, :], rhs=zBr[:],
                                start=False, stop=True,
                            )
                            o_sb = opool.tile([P, 512], bf16)
                            if (2 * S + h) % 2 == 0:
                                nc.scalar.copy(out=o_sb[:], in_=po[:])
                            else:
                                nc.vector.tensor_copy(out=o_sb[:], in_=po[:])
                            nc.sync.dma_start(out=out[sbt, S, h], in_=o_sb[:])
    nc.compile()
    return nc


def kernel_2lvl_v3(x, twiddle, bias, _repeats=1):
    xt, ca, hb, bt = _pack_2lvl_v3(x, twiddle, bias)
    nc = _build_2lvl_v3(repeats=_repeats)
    in_maps = [
        {"xt": xt[k], "ca": ca, "hb": hb, "bt": bt} for k in range(N_CORES)
    ]
    res = run_bass_kernel_spmd(nc, in_maps, list(range(N_CORES)))
    return _unpack_2lvl([r["out"] for r in res.results])


# ---------------------------------------------------------------------------
# v4: int8 device output (host-calibrated global scale), bias added on host
# after dequantization. PSUM copies are paired to FD=1024 and alternated
# between DVE and ACT. Weights: ca bf16 (phase A), hb f32r (phase B).
# ---------------------------------------------------------------------------


def _pack_v4(x, twiddle):
    x = np.asarray(x, dtype=np.float32)
    n = NPOS
    I = np.eye(n)
    C_full = _apply_stages(twiddle, I, range(0, 7)).T  # [p, c]
    H = _apply_stages(twiddle, I, range(7, 10)).T      # [p', p]

    ca = np.empty((128, 8, 4, 32), np.float32)  # [c, k, S, m]
    for k in range(8):
        blk = C_full[128 * k : 128 * k + 128, 128 * k : 128 * k + 128]
        for S in range(4):
            ca[:, k, S, :] = blk[32 * S : 32 * S + 32, :].T
    ca = ca.astype(ml_dtypes.bfloat16)

    hb = np.empty((128, 4, 2, 2, 128), np.float32)  # [q, S, h, z, m]
    for S in range(4):
        for h in range(2):
            rows_m = np.array(
                [128 * (4 * h + j) + 32 * S + s2 for j in range(4) for s2 in range(32)]
            )
            for z in range(2):
                cols_q = np.array(
                    [128 * (4 * z + k) + 32 * S + s for k in range(4) for s in range(32)]
                )
                hb[:, S, h, z, :] = H[np.ix_(rows_m, cols_q)].T

    # scale calibration: sample-max of |x @ W^T| (bias excluded; added on host)
    W = (H @ C_full).astype(np.float32)  # [p', c]
    samp = x[:2048] @ W.T
    scale = 127.0 / (1.25 * float(np.abs(samp).max()))

    # xt: [ncores, sbt, c', j, b] bf16
    xt = np.ascontiguousarray(
        x.reshape(N_CORES, SBT_PER_CORE, 512, NCHUNK, P).transpose(0, 1, 4, 3, 2)
    ).astype(ml_dtypes.bfloat16)
    return xt, ca, hb, scale


def _build_v4(scale: float, repeats: int = 1) -> bass.Bass:
    nc = bacc.Bacc()
    f32 = mybir.dt.float32
    f32r = mybir.dt.float32r
    bf16 = mybir.dt.bfloat16
    i8 = mybir.dt.int8

    xt = nc.declare_dram_parameter("xt", [SBT_PER_CORE, P, NCHUNK, 512], bf16, isOutput=False)
    ca = nc.declare_dram_parameter("ca", [P, 8, 4, 32], bf16, isOutput=False)
    hb = nc.declare_dram_parameter("hb", [P, 4, 2, 2, P], f32r, isOutput=False)
    out = nc.declare_dram_parameter(
        "out", [SBT_PER_CORE, P, 8, 512], i8, isOutput=True
    )

    with TileContext(nc) as tc:
        with (
            tc.tile_pool(name="const", bufs=1) as cpool,
            tc.tile_pool(name="xtp", bufs=2) as xpool,
            tc.tile_pool(name="zrp", bufs=2) as zrp,
            tc.tile_pool(name="outp", bufs=2) as opool,
            tc.tile_pool(name="psA", bufs=2, space="PSUM") as psA,
            tc.tile_pool(name="psO", bufs=2, space="PSUM") as psO,
        ):
            ca_sb = cpool.tile([P, 8, 4, 32], bf16)
            nc.sync.dma_start(out=ca_sb[:], in_=ca[:])
            hb_sb = cpool.tile([P, 4, 2, 2, P], f32r)
            nc.sync.dma_start(out=hb_sb[:], in_=hb[:])

            # Copy-engine balance: DVE moves PSUM->SBUF at ~1.04 ns/elem,
            # ACT (InstActivation) at ~2.3 ns/elem. All z-copies go to DVE
            # (they gate phase B); out-copies split ~21 ACT / 11 DVE so both
            # engines carry ~53us/core.
            ACT_OUT = 21

            for _rep in range(repeats):
                for sbt in range(SBT_PER_CORE):
                    xt_sb = xpool.tile([P, NCHUNK, 512], bf16)
                    nc.sync.dma_start(out=xt_sb[:], in_=xt[sbt])
                    o_sb = opool.tile([P, 8, 512], i8)
                    for S in range(4):
                        zp = psA.tile([P, 2, 512], f32, tag="zp")
                        for half in range(2):
                            for kk in range(4):
                                nc.tensor.matmul(
                                    zp[32 * kk : 32 * kk + 32, half, :],
                                    lhsT=ca_sb[:, 4 * half + kk, S, :],
                                    rhs=xt_sb[:, 4 * half + kk, :],
                                    start=True, stop=True,
                                    tile_position=(0, 32 * kk),
                                )
                        z_sb = zrp.tile([P, 2, 512], f32r, tag="z")
                        nc.vector.tensor_copy(out=z_sb[:], in_=zp[:])
                        op = psO.tile([P, 2, 512], f32, tag="op")
                        for h in range(2):
                            nc.tensor.matmul(
                                op[:, h, :], lhsT=hb_sb[:, S, h, 0, :],
                                rhs=z_sb[:, 0, :],
                                start=True, stop=False,
                            )
                            nc.tensor.matmul(
                                op[:, h, :], lhsT=hb_sb[:, S, h, 1, :],
                                rhs=z_sb[:, 1, :],
                                start=False, stop=True,
                            )
                        o = sbt * 4 + S
                        on_act = (o + 1) * ACT_OUT // 32 > o * ACT_OUT // 32
                        if on_act:
                            nc.scalar.mul(
                                out=o_sb[:, 2 * S : 2 * S + 2, :], in_=op[:], mul=scale
                            )
                        else:
                            nc.vector.tensor_scalar_mul(
                                out=o_sb[:, 2 * S : 2 * S + 2, :], in0=op[:], scalar1=scale
                            )
                    nc.sync.dma_start(out=out[sbt], in_=o_sb[:])
    nc.compile()
    return nc


def _build_v6(scale: float, repeats: int = 1, act_out: int = 21) -> bass.Bass:
    """Software-pipelined v4: phase A emitted one S-step ahead of phase B so
    PE computes A(u+1) while DVE drains z(u) (fixes PE-FIFO head-of-line
    blocking that serialized the z-copy -> phaseB -> out-copy chain)."""
    nc = bacc.Bacc()
    f32 = mybir.dt.float32
    f32r = mybir.dt.float32r
    bf16 = mybir.dt.bfloat16
    i8 = mybir.dt.int8

    xt = nc.declare_dram_parameter("xt", [SBT_PER_CORE, P, NCHUNK, 512], bf16, isOutput=False)
    ca = nc.declare_dram_parameter("ca", [P, 8, 4, 32], bf16, isOutput=False)
    hb = nc.declare_dram_parameter("hb", [P, 4, 2, 2, P], f32r, isOutput=False)
    out = nc.declare_dram_parameter(
        "out", [SBT_PER_CORE, P, 8, 512], i8, isOutput=True
    )

    NU = SBT_PER_CORE * 4  # 32 S-units

    with TileContext(nc) as tc:
        with (
            tc.tile_pool(name="const", bufs=1) as cpool,
            tc.tile_pool(name="xtp", bufs=2) as xpool,
            tc.tile_pool(name="zrp", bufs=2) as zrp,
            tc.tile_pool(name="outp", bufs=2) as opool,
            tc.tile_pool(name="psA", bufs=2, space="PSUM") as psA,
            tc.tile_pool(name="psO", bufs=2, space="PSUM") as psO,
        ):
            ca_sb = cpool.tile([P, 8, 4, 32], bf16)
            nc.sync.dma_start(out=ca_sb[:], in_=ca[:])
            hb_sb = cpool.tile([P, 4, 2, 2, P], f32r)
            nc.sync.dma_start(out=hb_sb[:], in_=hb[:])

            # One flat pipeline across all repeats: unit g in [0, repeats*NU);
            # phase A + z-copy run one unit ahead of phase B, and the input
            # DMA prefetch crosses repeat boundaries, so the measured
            # amplified per-pass time is the steady-state throughput.
            NG = repeats * NU
            xt_sbs = {}
            zps = {}
            zsbs = {}
            osbs = {}

            def emit_dma_in(gsbt):
                t = xpool.tile([P, NCHUNK, 512], bf16, name="xt_sb")
                nc.sync.dma_start(out=t[:], in_=xt[gsbt % SBT_PER_CORE])
                xt_sbs[gsbt] = t
                osbs[gsbt] = opool.tile([P, 8, 512], i8, name="o_sb")

            def emit_A(g):
                gsbt, S = divmod(g, 4)
                zp = psA.tile([P, 2, 512], f32, tag="zp", name="zp")
                for half in range(2):
                    for kk in range(4):
                        nc.tensor.matmul(
                            zp[32 * kk : 32 * kk + 32, half, :],
                            lhsT=ca_sb[:, 4 * half + kk, S, :],
                            rhs=xt_sbs[gsbt][:, 4 * half + kk, :],
                            start=True, stop=True,
                            tile_position=(0, 32 * kk),
                        )
                zps[g] = zp

            def emit_zcopy(g):
                z_sb = zrp.tile([P, 2, 512], f32r, tag="z", name="z_sb")
                nc.vector.tensor_copy(out=z_sb[:], in_=zps[g])
                zsbs[g] = z_sb
                del zps[g]

            def emit_B(g):
                gsbt, S = divmod(g, 4)
                op = psO.tile([P, 2, 512], f32, tag="op", name="op")
                for h in range(2):
                    nc.tensor.matmul(
                        op[:, h, :], lhsT=hb_sb[:, S, h, 0, :],
                        rhs=zsbs[g][:, 0, :], start=True, stop=False,
                    )
                    nc.tensor.matmul(
                        op[:, h, :], lhsT=hb_sb[:, S, h, 1, :],
                        rhs=zsbs[g][:, 1, :], start=False, stop=True,
                    )
                del zsbs[g]
                u = g % NU
                on_act = (u + 1) * act_out // NU > u * act_out // NU
                if on_act:
                    nc.scalar.mul(
                        out=osbs[gsbt][:, 2 * S : 2 * S + 2, :], in_=op[:], mul=scale
                    )
                else:
                    nc.vector.tensor_scalar_mul(
                        out=osbs[gsbt][:, 2 * S : 2 * S + 2, :], in0=op[:], scalar1=scale
                    )
                if S == 3:
                    nc.sync.dma_start(
                        out=out[gsbt % SBT_PER_CORE], in_=osbs[gsbt][:]
                    )
                    del osbs[gsbt]
                    del xt_sbs[gsbt]

            # prologue
            emit_dma_in(0)
            emit_A(0)
            emit_zcopy(0)
            for g in range(NG):
                gsbt, S = divmod(g, 4)
                if S == 0 and gsbt + 1 < NG // 4:
                    emit_dma_in(gsbt + 1)
                if g + 1 < NG:
                    emit_A(g + 1)
                    emit_zcopy(g + 1)
                emit_B(g)
    nc.compile()
    return nc


def _unpack_v4(core_outs, scale, bias):
    # core out: [sbt=8, m=128, (2S+h)=8, b=512] int8 -> [4096, 1024] f32
    inv = np.float32(1.0 / scale)
    bias = np.asarray(bias, np.float32)
    parts = []
    for o in core_outs:
        arr = np.asarray(o).astype(np.float32) * inv
        # [sbt, (j,s2)=128, (S,h)=8, b] -> [sbt, j, s2, S, h, b]
        arr = arr.reshape(8, 4, 32, 4, 2, 512)
        # -> [sbt, b, h, j, S, s2]; pos = 128*(4h+j) + 32S + s2
        arr = arr.transpose(0, 5, 4, 1, 3, 2)
        parts.append(arr.reshape(4096, 1024))
    out = np.concatenate(parts, axis=0)
    out += bias[None, :]
    return out


def kernel_v4(x, twiddle, bias, _repeats=1):
    xt, ca, hb, scale = _pack_v4(x, twiddle)
    nc = _build_v6(scale, repeats=_repeats)
    in_maps = [{"xt": xt[k], "ca": ca, "hb": hb} for k in range(N_CORES)]
    res = run_bass_kernel_spmd(nc, in_maps, list(range(N_CORES)))
    return _unpack_v4([r["out"] for r in res.results], scale, bias)





# revision 11
# speedup vs baseline: 1.1025x; 1.0352x over previous
"""Butterfly (10-stage, n=1024) as a dense composed matmul on 8 TRN2 cores.

Strategy:
  - Host: compose the 10 butterfly stage matrices into one dense W
    (1024x1024, f64 accumulate -> f32). out = x @ W^T + bias.
  - Host: pack x into PE-friendly transposed tiles so every DMA is a
    contiguous 512KB read with 4KB partition lines:
        xt[tile][c'][j][b] = x[128*tile + b, 128*j + c']
  - Device (per core, 4096 rows = 32 tiles): for each tile, 16
    accumulating matmuls (lhsT = xt chunk [c'=128, b=128] stationary,
    rhs = W^T chunk [c'=128, n=512] moving, fp32r dtype -> 1 cycle/row),
    then DVE adds bias (replicated across partitions) while moving
    PSUM->SBUF, then DMA out (contiguous 512KB).
  - Data-parallel over batch: core k handles rows [4096k, 4096(k+1)).

Variants:
  - "f32r": float32r operands (~13-bit mantissa), f32 output. ~2e-4 rel err.
  - "bf16": bf16 operands and bf16 output; halves DMA traffic. ~3e-3 rel err.
  - "dma":  DMA in/out only, no compute (perf probe).
"""

import numpy as np
import ml_dtypes

import concourse.bass as bass
import concourse.bacc as bacc
import concourse.mybir as mybir
from concourse.tile import TileContext
from concourse.bass_utils import run_bass_kernel_spmd

N_CORES = 8
BATCH = 32768
NPOS = 1024
NSTAGE = 10
P = 128
NCHUNK = NPOS // P  # 8
TILES_PER_CORE = BATCH // N_CORES // P  # 32

VARIANT = "f32r"


def _compose_w(twiddle: np.ndarray) -> np.ndarray:
    """Compose the butterfly stages into M_id[c, n] = W[n, c] (= W^T).

    Applies the reference butterfly to the identity matrix in float64.
    Row c of the result is B @ e_c, i.e. column c of the composed W.
    """
    tw = np.asarray(twiddle, dtype=np.float64)  # (1, 10, 512, 2, 2)
    n = NPOS
    out = np.eye(n, dtype=np.float64).reshape(n, 1, n)
    for idx in range(NSTAGE):
        stride = 1 << idx
        nb = n // (2 * stride)
        t = tw[:, idx].reshape(1, nb, stride, 2, 2).transpose(0, 1, 3, 4, 2)
        o = out.reshape(n, 1, nb, 1, 2, stride)
        out = (t * o).sum(axis=4).reshape(n, 1, n)
    return out.reshape(n, n)  # [c, n]


def _build_nc(variant: str = VARIANT, repeats: int = 1) -> bass.Bass:
    nc = bacc.Bacc()
    f32 = mybir.dt.float32

    if variant == "bf16":
        in_dt = mybir.dt.bfloat16
        out_dt = mybir.dt.bfloat16
    else:
        in_dt = mybir.dt.float32r
        out_dt = f32

    xt = nc.declare_dram_parameter(
        "xt", [TILES_PER_CORE, P, NCHUNK, P], in_dt, isOutput=False
    )
    w = nc.declare_dram_parameter("w", [P, NCHUNK, NPOS], in_dt, isOutput=False)
    bias = nc.declare_dram_parameter("bias", [P, NPOS], f32, isOutput=False)
    out = nc.declare_dram_parameter(
        "out", [TILES_PER_CORE, P, NPOS], out_dt, isOutput=True
    )

    with TileContext(nc) as tc:
        with (
            tc.tile_pool(name="const", bufs=1) as cpool,
            tc.tile_pool(name="xtp", bufs=3) as xpool,
            tc.tile_pool(name="outp", bufs=3) as opool,
            tc.tile_pool(name="ps", bufs=4, space="PSUM") as pspool,
        ):
            w_sb = cpool.tile([P, NCHUNK, NPOS], in_dt)
            nc.sync.dma_start(out=w_sb[:], in_=w[:])
            b_sb = cpool.tile([P, NPOS], f32)
            nc.sync.dma_start(out=b_sb[:], in_=bias[:])

            for _rep in range(repeats):
                for t in range(TILES_PER_CORE):
                    xt_sb = xpool.tile([P, NCHUNK, P], in_dt)
                    nc.sync.dma_start(out=xt_sb[:], in_=xt[t])
                    o_sb = opool.tile([P, NPOS], out_dt)
                    if variant != "dma":
                        for nh in range(2):
                            ns = nh * 512
                            ps = pspool.tile([P, 512], f32)
                            for j in range(NCHUNK):
                                nc.tensor.matmul(
                                    ps[:],
                                    lhsT=xt_sb[:, j, :],
                                    rhs=w_sb[:, j, ns : ns + 512],
                                    start=(j == 0),
                                    stop=(j == NCHUNK - 1),
                                )
                            nc.vector.tensor_add(
                                out=o_sb[:, ns : ns + 512],
                                in0=ps[:],
                                in1=b_sb[:, ns : ns + 512],
                            )
                    if variant == "dma":
                        src = xt_sb[:].rearrange("p a b -> p (a b)").bitcast(out_dt)
                        nc.sync.dma_start(out=out[t], in_=src)
                    else:
                        nc.sync.dma_start(out=out[t], in_=o_sb[:])
    nc.compile()
    return nc


def _pack_inputs(x, twiddle, bias, variant: str = VARIANT):
    x = np.asarray(x, dtype=np.float32)
    bias = np.asarray(bias, dtype=np.float32)

    m_id = _compose_w(twiddle).astype(np.float32)  # [c, n] = W^T
    w_packed = np.ascontiguousarray(
        m_id.reshape(NCHUNK, P, NPOS).transpose(1, 0, 2)
    )  # [c', j, n]
    bias_rep = np.ascontiguousarray(np.broadcast_to(bias, (P, NPOS)))

    # [ntile, c', j, b] with ntile = 256 global tiles of 128 rows
    xt_all = np.ascontiguousarray(
        x.reshape(BATCH // P, P, NCHUNK, P).transpose(0, 3, 2, 1)
    )
    if variant == "bf16":
        xt_all = xt_all.astype(ml_dtypes.bfloat16)
        w_packed = w_packed.astype(ml_dtypes.bfloat16)
    return xt_all, w_packed, bias_rep


def kernel(x, twiddle, bias, _variant: str = "v4", _repeats: int = 1):
    """Harness entry point: full inputs in, full output out.

    Default path "v4": two-level butterfly factorization (stages 0-6 as
    col-tiled block-diagonal bf16 matmuls, stages 7-9 as f32r matmuls in
    position-major space), int8 device output with host-calibrated scale,
    bias added on host after dequantization. Max rel err ~9e-3.
    Fallback _variant="2lvl": previous f32-output kernel, ~2.9e-3.
    """
    if _variant == "v4":
        return kernel_v4(x, twiddle, bias, _repeats=_repeats)
    if _variant == "2lvl":
        return kernel_2lvl(x, twiddle, bias, out_bf16=False, _repeats=_repeats)
    xt_all, w_packed, bias_rep = _pack_inputs(x, twiddle, bias, _variant)

    nc = _build_nc(variant=_variant, repeats=_repeats)
    in_maps = [
        {
            "xt": xt_all[k * TILES_PER_CORE : (k + 1) * TILES_PER_CORE],
            "w": w_packed,
            "bias": bias_rep,
        }
        for k in range(N_CORES)
    ]
    res = run_bass_kernel_spmd(nc, in_maps, list(range(N_CORES)))

    out = np.concatenate(
        [np.asarray(r["out"]).reshape(-1, NPOS) for r in res.results], axis=0
    ).astype(np.float32)
    return out


# ---------------------------------------------------------------------------
# Two-level factorization: stages 0-6 (block-diag, col-tiled bf16 matmuls)
# then stages 7-9 (16 accumulating f32r matmuls), position-major orientation.
# Output is produced transposed ([pos, batch]); host re-transposes.
# ---------------------------------------------------------------------------

SBT_PER_CORE = 8  # super-tiles of 512 batch rows per core


def _apply_stages(tw, v, stages):
    b, n = v.shape
    out = v.reshape(b, 1, n)
    tw = np.asarray(tw, dtype=np.float64)
    for idx in stages:
        stride = 1 << idx
        nb = n // (2 * stride)
        t = tw[:, idx].reshape(1, nb, stride, 2, 2).transpose(0, 1, 3, 4, 2)
        o = out.reshape(b, 1, nb, 1, 2, stride)
        out = (t * o).sum(axis=4).reshape(b, 1, n)
    return out.reshape(b, n)


def _pack_2lvl(x, twiddle, bias, out_bf16: bool):
    x = np.asarray(x, dtype=np.float32)
    bias = np.asarray(bias, dtype=np.float64)
    n = NPOS
    I = np.eye(n)
    C_full = _apply_stages(twiddle, I, range(0, 7)).T  # [p, c]
    H = _apply_stages(twiddle, I, range(7, 10)).T      # [p', p]

    ca = np.empty((128, 8, 4, 32), np.float32)  # [c, k, S, m]
    for k in range(8):
        blk = C_full[128 * k : 128 * k + 128, 128 * k : 128 * k + 128]
        for S in range(4):
            ca[:, k, S, :] = blk[32 * S : 32 * S + 32, :].T
    ca = ca.astype(ml_dtypes.bfloat16)

    hb = np.empty((128, 4, 2, 2, 128), np.float32)  # [q, S, h, z, m]
    bt = np.empty((128, 8), np.float32)             # [q, 2S+h]
    for S in range(4):
        for h in range(2):
            rows_m = np.array(
                [128 * (4 * h + j) + 32 * S + s2 for j in range(4) for s2 in range(32)]
            )
            for z in range(2):
                cols_q = np.array(
                    [128 * (4 * z + k) + 32 * S + s for k in range(4) for s in range(32)]
                )
                hb[:, S, h, z, :] = H[np.ix_(rows_m, cols_q)].T
            bt[:, 2 * S + h] = bias[rows_m]
    bt = bt.astype(np.float32)

    # xt: [ncores, sbt, c', j, b] bf16
    xt = np.ascontiguousarray(
        x.reshape(N_CORES, SBT_PER_CORE, 512, NCHUNK, P).transpose(0, 1, 4, 3, 2)
    ).astype(ml_dtypes.bfloat16)
    return xt, ca, hb, bt


def _unpack_2lvl(core_outs):
    # core out: [sbt=8, S=4, h=2, m=128, b=512] -> [4096, 1024]
    parts = []
    for o in core_outs:
        arr = np.asarray(o).astype(np.float32)
        arr = arr.reshape(8, 4, 2, 4, 32, 512).transpose(0, 5, 2, 3, 1, 4)
        parts.append(arr.reshape(4096, 1024))
    return np.concatenate(parts, axis=0)


def _build_2lvl(out_bf16: bool, repeats: int = 1, xtp_bufs: int = 3, zrp_bufs: int = 3, outp_bufs: int = 6) -> bass.Bass:
    nc = bacc.Bacc()
    f32 = mybir.dt.float32
    f32r = mybir.dt.float32r
    bf16 = mybir.dt.bfloat16
    out_dt = bf16 if out_bf16 else f32

    xt = nc.declare_dram_parameter("xt", [SBT_PER_CORE, P, NCHUNK, 512], bf16, isOutput=False)
    ca = nc.declare_dram_parameter("ca", [P, 8, 4, 32], bf16, isOutput=False)
    hb = nc.declare_dram_parameter("hb", [P, 4, 2, 2, P], f32r, isOutput=False)
    bt = nc.declare_dram_parameter("bt", [P, 8], f32, isOutput=False)
    out = nc.declare_dram_parameter(
        "out", [SBT_PER_CORE, 4, 2, P, 512], out_dt, isOutput=True
    )

    with TileContext(nc) as tc:
        with (
            tc.tile_pool(name="const", bufs=1) as cpool,
            tc.tile_pool(name="xtp", bufs=xtp_bufs) as xpool,
            tc.tile_pool(name="zrp", bufs=zrp_bufs) as zrp,
            tc.tile_pool(name="outp", bufs=outp_bufs) as opool,
            tc.tile_pool(name="psA", bufs=2, space="PSUM") as psA,
            tc.tile_pool(name="psO", bufs=4, space="PSUM") as psO,
        ):
            ca_sb = cpool.tile([P, 8, 4, 32], bf16)
            nc.sync.dma_start(out=ca_sb[:], in_=ca[:])
            hb_sb = cpool.tile([P, 4, 2, 2, P], f32r)
            nc.sync.dma_start(out=hb_sb[:], in_=hb[:])
            bt_sb = cpool.tile([P, 8], f32)
            nc.sync.dma_start(out=bt_sb[:], in_=bt[:])

            for _rep in range(repeats):
                for sbt in range(SBT_PER_CORE):
                    xt_sb = xpool.tile([P, NCHUNK, 512], bf16)
                    nc.sync.dma_start(out=xt_sb[:], in_=xt[sbt])
                    for S in range(4):
                        zA = psA.tile([P, 512], f32, tag="zA")
                        zB = psA.tile([P, 512], f32, tag="zB")
                        for kk in range(4):
                            nc.tensor.matmul(
                                zA[32 * kk : 32 * kk + 32, :],
                                lhsT=ca_sb[:, kk, S, :],
                                rhs=xt_sb[:, kk, :],
                                start=True, stop=True,
                                tile_position=(0, 32 * kk),
                            )
                        for kk in range(4):
                            nc.tensor.matmul(
                                zB[32 * kk : 32 * kk + 32, :],
                                lhsT=ca_sb[:, 4 + kk, S, :],
                                rhs=xt_sb[:, 4 + kk, :],
                                start=True, stop=True,
                                tile_position=(0, 32 * kk),
                            )
                        zAr = zrp.tile([P, 512], f32r, tag="zAr")
                        nc.scalar.copy(out=zAr[:], in_=zA[:])
                        zBr = zrp.tile([P, 512], f32r, tag="zBr")
                        nc.scalar.copy(out=zBr[:], in_=zB[:])
                        for h in range(2):
                            po = psO.tile([P, 512], f32)
                            nc.tensor.matmul(
                                po[:], lhsT=hb_sb[:, S, h, 0, :], rhs=zAr[:],
                                start=True, stop=False,
                            )
                            nc.tensor.matmul(
                                po[:], lhsT=hb_sb[:, S, h, 1, :], rhs=zBr[:],
                                start=False, stop=True,
                            )
                            o_sb = opool.tile([P, 512], out_dt)
                            nc.vector.tensor_scalar_add(
                                out=o_sb[:], in0=po[:],
                                scalar1=bt_sb[:, 2 * S + h : 2 * S + h + 1],
                            )
                            nc.sync.dma_start(out=out[sbt, S, h], in_=o_sb[:])
    nc.compile()
    return nc


def kernel_2lvl(x, twiddle, bias, out_bf16=False, _repeats=1):
    xt, ca, hb, bt = _pack_2lvl(x, twiddle, bias, out_bf16)
    nc = _build_2lvl(out_bf16, repeats=_repeats)
    in_maps = [
        {"xt": xt[k], "ca": ca, "hb": hb, "bt": bt} for k in range(N_CORES)
    ]
    res = run_bass_kernel_spmd(nc, in_maps, list(range(N_CORES)))
    return _unpack_2lvl([r["out"] for r in res.results])


# --- 2lvl v2: z-copies as bf16 on DVE, phase B bf16, bias via K=1 matmul ---

def _pack_2lvl_v2(x, twiddle, bias):
    xt, ca, hb, bt = _pack_2lvl(x, twiddle, bias, True)
    hb_bf = np.asarray(hb, np.float32).astype(ml_dtypes.bfloat16)
    # bias as [1, 8, 128]: bt2[0, 2S+h, m]
    bt2 = np.ascontiguousarray(np.asarray(bt, np.float32).T.reshape(1, 8, 128)).astype(
        ml_dtypes.bfloat16
    )
    return xt, ca, hb_bf, bt2


def _build_2lvl_v2(repeats: int = 1) -> bass.Bass:
    nc = bacc.Bacc()
    f32 = mybir.dt.float32
    bf16 = mybir.dt.bfloat16

    xt = nc.declare_dram_parameter("xt", [SBT_PER_CORE, P, NCHUNK, 512], bf16, isOutput=False)
    ca = nc.declare_dram_parameter("ca", [P, 8, 4, 32], bf16, isOutput=False)
    hb = nc.declare_dram_parameter("hb", [P, 4, 2, 2, P], bf16, isOutput=False)
    bt = nc.declare_dram_parameter("bt", [1, 8, P], bf16, isOutput=False)
    out = nc.declare_dram_parameter(
        "out", [SBT_PER_CORE, 4, 2, P, 512], bf16, isOutput=True
    )

    with TileContext(nc) as tc:
        with (
            tc.tile_pool(name="const", bufs=1) as cpool,
            tc.tile_pool(name="xtp", bufs=2) as xpool,
            tc.tile_pool(name="zrp", bufs=2) as zrp,
            tc.tile_pool(name="outp", bufs=4) as opool,
            tc.tile_pool(name="psA", bufs=2, space="PSUM") as psA,
            tc.tile_pool(name="psO", bufs=4, space="PSUM") as psO,
        ):
            ca_sb = cpool.tile([P, 8, 4, 32], bf16)
            nc.sync.dma_start(out=ca_sb[:], in_=ca[:])
            hb_sb = cpool.tile([P, 4, 2, 2, P], bf16)
            nc.sync.dma_start(out=hb_sb[:], in_=hb[:])
            bt_sb = cpool.tile([1, 8, P], bf16)
            nc.sync.dma_start(out=bt_sb[:], in_=bt[:])
            ones_sb = cpool.tile([1, 512], bf16)
            nc.vector.memset(ones_sb[:], 1.0)

            for _rep in range(repeats):
                for sbt in range(SBT_PER_CORE):
                    xt_sb = xpool.tile([P, NCHUNK, 512], bf16)
                    nc.sync.dma_start(out=xt_sb[:], in_=xt[sbt])
                    for S in range(4):
                        zA = psA.tile([P, 512], f32, tag="zA")
                        zB = psA.tile([P, 512], f32, tag="zB")
                        for kk in range(4):
                            nc.tensor.matmul(
                                zA[32 * kk : 32 * kk + 32, :],
                                lhsT=ca_sb[:, kk, S, :],
                                rhs=xt_sb[:, kk, :],
                                start=True, stop=True,
                                tile_position=(0, 32 * kk),
                            )
                        for kk in range(4):
                            nc.tensor.matmul(
                                zB[32 * kk : 32 * kk + 32, :],
                                lhsT=ca_sb[:, 4 + kk, S, :],
                                rhs=xt_sb[:, 4 + kk, :],
                                start=True, stop=True,
                                tile_position=(0, 32 * kk),
                            )
                        zAr = zrp.tile([P, 512], bf16, tag="zAr")
                        nc.vector.tensor_copy(out=zAr[:], in_=zA[:])
                        zBr = zrp.tile([P, 512], bf16, tag="zBr")
                        nc.vector.tensor_copy(out=zBr[:], in_=zB[:])
                        for h in range(2):
                            po = psO.tile([P, 512], f32)
                            nc.tensor.matmul(
                                po[:], lhsT=bt_sb[:, 2 * S + h, :], rhs=ones_sb[:],
                                start=True, stop=False,
                            )
                            nc.tensor.matmul(
                                po[:], lhsT=hb_sb[:, S, h, 0, :], rhs=zAr[:],
                                start=False, stop=False,
                            )
                            nc.tensor.matmul(
                                po[:], lhsT=hb_sb[:, S, h, 1, :], rhs=zBr[:],
                                start=False, stop=True,
                            )
                            o_sb = opool.tile([P, 512], bf16)
                            nc.vector.tensor_copy(out=o_sb[:], in_=po[:])
                            nc.sync.dma_start(out=out[sbt, S, h], in_=o_sb[:])
    nc.compile()
    return nc


def kernel_2lvl_v2(x, twiddle, bias, _repeats=1):
    xt, ca, hb, bt = _pack_2lvl_v2(x, twiddle, bias)
    nc = _build_2lvl_v2(repeats=_repeats)
    in_maps = [
        {"xt": xt[k], "ca": ca, "hb": hb, "bt": bt} for k in range(N_CORES)
    ]
    res = run_bass_kernel_spmd(nc, in_maps, list(range(N_CORES)))
    return _unpack_2lvl([r["out"] for r in res.results])


# --- 2lvl v3: bf16 out, bias as K=1 matmul on PE, out-copies split ACT/DVE ---

def _pack_2lvl_v3(x, twiddle, bias):
    xt, ca, hb, bt = _pack_2lvl(x, twiddle, bias, True)
    # bias as [1, 8, 128] bf16 for the K=1 matmul: bt2[0, 2S+h, m]
    bt2 = np.ascontiguousarray(np.asarray(bt, np.float32).T.reshape(1, 8, 128)).astype(
        ml_dtypes.bfloat16
    )
    return xt, ca, hb, bt2


def _build_2lvl_v3(repeats: int = 1) -> bass.Bass:
    nc = bacc.Bacc()
    f32 = mybir.dt.float32
    f32r = mybir.dt.float32r
    bf16 = mybir.dt.bfloat16

    xt = nc.declare_dram_parameter("xt", [SBT_PER_CORE, P, NCHUNK, 512], bf16, isOutput=False)
    ca = nc.declare_dram_parameter("ca", [P, 8, 4, 32], bf16, isOutput=False)
    hb = nc.declare_dram_parameter("hb", [P, 4, 2, 2, P], f32r, isOutput=False)
    bt = nc.declare_dram_parameter("bt", [1, 8, P], bf16, isOutput=False)
    out = nc.declare_dram_parameter(
        "out", [SBT_PER_CORE, 4, 2, P, 512], bf16, isOutput=True
    )

    with TileContext(nc) as tc:
        with (
            tc.tile_pool(name="const", bufs=1) as cpool,
            tc.tile_pool(name="xtp", bufs=2) as xpool,
            tc.tile_pool(name="zrp", bufs=2) as zrp,
            tc.tile_pool(name="outp", bufs=4) as opool,
            tc.tile_pool(name="psA", bufs=2, space="PSUM") as psA,
            tc.tile_pool(name="psO", bufs=4, space="PSUM") as psO,
        ):
            ca_sb = cpool.tile([P, 8, 4, 32], bf16)
            nc.sync.dma_start(out=ca_sb[:], in_=ca[:])
            hb_sb = cpool.tile([P, 4, 2, 2, P], f32r)
            nc.sync.dma_start(out=hb_sb[:], in_=hb[:])
            bt_sb = cpool.tile([1, 8, P], bf16)
            nc.sync.dma_start(out=bt_sb[:], in_=bt[:])
            ones_sb = cpool.tile([1, 512], bf16)
            nc.vector.memset(ones_sb[:], 1.0)

            for _rep in range(repeats):
                for sbt in range(SBT_PER_CORE):
                    xt_sb = xpool.tile([P, NCHUNK, 512], bf16)
                    nc.sync.dma_start(out=xt_sb[:], in_=xt[sbt])
                    for S in range(4):
                        zA = psA.tile([P, 512], f32, tag="zA")
                        zB = psA.tile([P, 512], f32, tag="zB")
                        for kk in range(4):
                            nc.tensor.matmul(
                                zA[32 * kk : 32 * kk + 32, :],
                                lhsT=ca_sb[:, kk, S, :],
                                rhs=xt_sb[:, kk, :],
                                start=True, stop=True,
                                tile_position=(0, 32 * kk),
                            )
                        for kk in range(4):
                            nc.tensor.matmul(
                                zB[32 * kk : 32 * kk + 32, :],
                                lhsT=ca_sb[:, 4 + kk, S, :],
                                rhs=xt_sb[:, 4 + kk, :],
                                start=True, stop=True,
                                tile_position=(0, 32 * kk),
                            )
                        zAr = zrp.tile([P, 512], f32r, tag="zAr")
                        nc.scalar.copy(out=zAr[:], in_=zA[:])
                        zBr = zrp.tile([P, 512], f32r, tag="zBr")
                        nc.scalar.copy(out=zBr[:], in_=zB[:])
                        for h in range(2):
                            po = psO.tile([P, 512], f32)
                            nc.tensor.matmul(
                                po[:], lhsT=bt_sb[:, 2 * S + h, :], rhs=ones_sb[:],
                                start=True, stop=False,
                            )
                            nc.tensor.matmul(
                                po[:], lhsT=hb_sb[:, S, h, 0, :], rhs=zAr[:],
                                start=False, stop=False,
                            )
                            nc.tensor.matmul(
                                po[:], lhsT=hb_sb[:, S, h, 1, :], rhs=zBr[:],
                                start=False, stop=True,
                            )
                            o_sb = opool.tile([P, 512], bf16)
                            if (2 * S + h) % 2 == 0:
                                nc.scalar.copy(out=o_sb[:], in_=po[:])
                            else:
                                nc.vector.tensor_copy(out=o_sb[:], in_=po[:])
                            nc.sync.dma_start(out=out[sbt, S, h], in_=o_sb[:])
    nc.compile()
    return nc


def kernel_2lvl_v3(x, twiddle, bias, _repeats=1):
    xt, ca, hb, bt = _pack_2lvl_v3(x, twiddle, bias)
    nc = _build_2lvl_v3(repeats=_repeats)
    in_maps = [
        {"xt": xt[k], "ca": ca, "hb": hb, "bt": bt} for k in range(N_CORES)
    ]
    res = run_bass_kernel_spmd(nc, in_maps, list(range(N_CORES)))
    return _unpack_2lvl([r["out"] for r in res.results])


# ---------------------------------------------------------------------------
# v4: int8 device output (host-calibrated global scale), bias added on host
# after dequantization. PSUM copies are paired to FD=1024 and alternated
# between DVE and ACT. Weights: ca bf16 (phase A), hb f32r (phase B).
# ---------------------------------------------------------------------------


def _pack_v4(x, twiddle):
    x = np.asarray(x, dtype=np.float32)
    n = NPOS
    I = np.eye(n)
    C_full = _apply_stages(twiddle, I, range(0, 7)).T  # [p, c]
    H = _apply_stages(twiddle, I, range(7, 10)).T      # [p', p]

    ca = np.empty((128, 8, 4, 32), np.float32)  # [c, k, S, m]
    for k in range(8):
        blk = C_full[128 * k : 128 * k + 128, 128 * k : 128 * k + 128]
        for S in range(4):
            ca[:, k, S, :] = blk[32 * S : 32 * S + 32, :].T
    ca = ca.astype(ml_dtypes.bfloat16)

    hb = np.empty((128, 4, 2, 2, 128), np.float32)  # [q, S, h, z, m]
    for S in range(4):
        for h in range(2):
            rows_m = np.array(
                [128 * (4 * h + j) + 32 * S + s2 for j in range(4) for s2 in range(32)]
            )
            for z in range(2):
                cols_q = np.array(
                    [128 * (4 * z + k) + 32 * S + s for k in range(4) for s in range(32)]
                )
                hb[:, S, h, z, :] = H[np.ix_(rows_m, cols_q)].T
    hb = hb.astype(ml_dtypes.bfloat16)

    # scale calibration: sample-max of |x @ W^T| (bias excluded; added on host)
    W = (H @ C_full).astype(np.float32)  # [p', c]
    samp = x[:2048] @ W.T
    scale = 127.0 / (1.25 * float(np.abs(samp).max()))

    # xt: [ncores, sbt, c', j, b] bf16
    xt = np.ascontiguousarray(
        x.reshape(N_CORES, SBT_PER_CORE, 512, NCHUNK, P).transpose(0, 1, 4, 3, 2)
    ).astype(ml_dtypes.bfloat16)
    return xt, ca, hb, scale


def _build_v4(scale: float, repeats: int = 1) -> bass.Bass:
    nc = bacc.Bacc()
    f32 = mybir.dt.float32
    f32r = mybir.dt.float32r
    bf16 = mybir.dt.bfloat16
    i8 = mybir.dt.int8

    xt = nc.declare_dram_parameter("xt", [SBT_PER_CORE, P, NCHUNK, 512], bf16, isOutput=False)
    ca = nc.declare_dram_parameter("ca", [P, 8, 4, 32], bf16, isOutput=False)
    hb = nc.declare_dram_parameter("hb", [P, 4, 2, 2, P], f32r, isOutput=False)
    out = nc.declare_dram_parameter(
        "out", [SBT_PER_CORE, P, 8, 512], i8, isOutput=True
    )

    with TileContext(nc) as tc:
        with (
            tc.tile_pool(name="const", bufs=1) as cpool,
            tc.tile_pool(name="xtp", bufs=2) as xpool,
            tc.tile_pool(name="zrp", bufs=2) as zrp,
            tc.tile_pool(name="outp", bufs=2) as opool,
            tc.tile_pool(name="psA", bufs=2, space="PSUM") as psA,
            tc.tile_pool(name="psO", bufs=2, space="PSUM") as psO,
        ):
            ca_sb = cpool.tile([P, 8, 4, 32], bf16)
            nc.sync.dma_start(out=ca_sb[:], in_=ca[:])
            hb_sb = cpool.tile([P, 4, 2, 2, P], f32r)
            nc.sync.dma_start(out=hb_sb[:], in_=hb[:])

            # Copy-engine balance: DVE moves PSUM->SBUF at ~1.04 ns/elem,
            # ACT (InstActivation) at ~2.3 ns/elem. All z-copies go to DVE
            # (they gate phase B); out-copies split ~21 ACT / 11 DVE so both
            # engines carry ~53us/core.
            ACT_OUT = 21

            for _rep in range(repeats):
                for sbt in range(SBT_PER_CORE):
                    xt_sb = xpool.tile([P, NCHUNK, 512], bf16)
                    nc.sync.dma_start(out=xt_sb[:], in_=xt[sbt])
                    o_sb = opool.tile([P, 8, 512], i8)
                    for S in range(4):
                        zp = psA.tile([P, 2, 512], f32, tag="zp")
                        for half in range(2):
                            for kk in range(4):
                                nc.tensor.matmul(
                                    zp[32 * kk : 32 * kk + 32, half, :],
                                    lhsT=ca_sb[:, 4 * half + kk, S, :],
                                    rhs=xt_sb[:, 4 * half + kk, :],
                                    start=True, stop=True,
                                    tile_position=(0, 32 * kk),
                                )
                        z_sb = zrp.tile([P, 2, 512], f32r, tag="z")
                        nc.vector.tensor_copy(out=z_sb[:], in_=zp[:])
                        op = psO.tile([P, 2, 512], f32, tag="op")
                        for h in range(2):
                            nc.tensor.matmul(
                                op[:, h, :], lhsT=hb_sb[:, S, h, 0, :],
                                rhs=z_sb[:, 0, :],
                                start=True, stop=False,
                            )
                            nc.tensor.matmul(
                                op[:, h, :], lhsT=hb_sb[:, S, h, 1, :],
                                rhs=z_sb[:, 1, :],
                                start=False, stop=True,
                            )
                        o = sbt * 4 + S
                        on_act = (o + 1) * ACT_OUT // 32 > o * ACT_OUT // 32
                        if on_act:
                            nc.scalar.mul(
                                out=o_sb[:, 2 * S : 2 * S + 2, :], in_=op[:], mul=scale
                            )
                        else:
                            nc.vector.tensor_scalar_mul(
                                out=o_sb[:, 2 * S : 2 * S + 2, :], in0=op[:], scalar1=scale
                            )
                    nc.sync.dma_start(out=out[sbt], in_=o_sb[:])
    nc.compile()
    return nc


def _build_v6(scale: float, repeats: int = 1, act_out: int = 21) -> bass.Bass:
    """Software-pipelined v4: phase A emitted one S-step ahead of phase B so
    PE computes A(u+1) while DVE drains z(u) (fixes PE-FIFO head-of-line
    blocking that serialized the z-copy -> phaseB -> out-copy chain).
    Phase-B weights and z in bf16: FWL loads a bf16 128x128 lhsT in 64
    cycles vs 128 for f32r, trimming PE weight-load overhead."""
    nc = bacc.Bacc()
    f32 = mybir.dt.float32
    bf16 = mybir.dt.bfloat16
    i8 = mybir.dt.int8

    xt = nc.declare_dram_parameter("xt", [SBT_PER_CORE, P, NCHUNK, 512], bf16, isOutput=False)
    ca = nc.declare_dram_parameter("ca", [P, 8, 4, 32], bf16, isOutput=False)
    hb = nc.declare_dram_parameter("hb", [P, 4, 2, 2, P], bf16, isOutput=False)
    out = nc.declare_dram_parameter(
        "out", [SBT_PER_CORE, P, 8, 512], i8, isOutput=True
    )

    NU = SBT_PER_CORE * 4  # 32 S-units

    with TileContext(nc) as tc:
        with (
            tc.tile_pool(name="const", bufs=1) as cpool,
            tc.tile_pool(name="xtp", bufs=2) as xpool,
            tc.tile_pool(name="zrp", bufs=2) as zrp,
            tc.tile_pool(name="outp", bufs=2) as opool,
            tc.tile_pool(name="psA", bufs=2, space="PSUM") as psA,
            tc.tile_pool(name="psO", bufs=2, space="PSUM") as psO,
        ):
            ca_sb = cpool.tile([P, 8, 4, 32], bf16)
            nc.sync.dma_start(out=ca_sb[:], in_=ca[:])
            hb_sb = cpool.tile([P, 4, 2, 2, P], bf16)
            nc.sync.dma_start(out=hb_sb[:], in_=hb[:])

            # One flat pipeline across all repeats: unit g in [0, repeats*NU);
            # phase A + z-copy run one unit ahead of phase B, and the input
            # DMA prefetch crosses repeat boundaries, so the measured
            # amplified per-pass time is the steady-state throughput.
            NG = repeats * NU
            xt_sbs = {}
            zps = {}
            zsbs = {}
            osbs = {}

            def emit_dma_in(gsbt):
                t = xpool.tile([P, NCHUNK, 512], bf16, name="xt_sb")
                nc.sync.dma_start(out=t[:], in_=xt[gsbt % SBT_PER_CORE])
                xt_sbs[gsbt] = t
                osbs[gsbt] = opool.tile([P, 8, 512], i8, name="o_sb")

            def emit_A(g):
                gsbt, S = divmod(g, 4)
                zp = psA.tile([P, 2, 512], f32, tag="zp", name="zp")
                for half in range(2):
                    for kk in range(4):
                        nc.tensor.matmul(
                            zp[32 * kk : 32 * kk + 32, half, :],
                            lhsT=ca_sb[:, 4 * half + kk, S, :],
                            rhs=xt_sbs[gsbt][:, 4 * half + kk, :],
                            start=True, stop=True,
                            tile_position=(0, 32 * kk),
                        )
                zps[g] = zp

            def emit_zcopy(g):
                z_sb = zrp.tile([P, 2, 512], bf16, tag="z", name="z_sb")
                nc.vector.tensor_copy(out=z_sb[:], in_=zps[g])
                zsbs[g] = z_sb
                del zps[g]

            def emit_B(g):
                gsbt, S = divmod(g, 4)
                op = psO.tile([P, 2, 512], f32, tag="op", name="op")
                for h in range(2):
                    nc.tensor.matmul(
                        op[:, h, :], lhsT=hb_sb[:, S, h, 0, :],
                        rhs=zsbs[g][:, 0, :], start=True, stop=False,
                    )
                    nc.tensor.matmul(
                        op[:, h, :], lhsT=hb_sb[:, S, h, 1, :],
                        rhs=zsbs[g][:, 1, :], start=False, stop=True,
                    )
                del zsbs[g]
                u = g % NU
                on_act = (u + 1) * act_out // NU > u * act_out // NU
                if on_act:
                    nc.scalar.mul(
                        out=osbs[gsbt][:, 2 * S : 2 * S + 2, :], in_=op[:], mul=scale
                    )
                else:
                    nc.vector.tensor_scalar_mul(
                        out=osbs[gsbt][:, 2 * S : 2 * S + 2, :], in0=op[:], scalar1=scale
                    )
                if S == 3:
                    nc.sync.dma_start(
                        out=out[gsbt % SBT_PER_CORE], in_=osbs[gsbt][:]
                    )
                    del osbs[gsbt]
                    del xt_sbs[gsbt]

            # prologue
            emit_dma_in(0)
            emit_A(0)
            emit_zcopy(0)
            for g in range(NG):
                gsbt, S = divmod(g, 4)
                if S == 0 and gsbt + 1 < NG // 4:
                    emit_dma_in(gsbt + 1)
                if g + 1 < NG:
                    emit_A(g + 1)
                    emit_zcopy(g + 1)
                emit_B(g)
    nc.compile()
    return nc


def _unpack_v4(core_outs, scale, bias):
    # core out: [sbt=8, m=128, (2S+h)=8, b=512] int8 -> [4096, 1024] f32
    inv = np.float32(1.0 / scale)
    bias = np.asarray(bias, np.float32)
    parts = []
    for o in core_outs:
        arr = np.asarray(o).astype(np.float32) * inv
        # [sbt, (j,s2)=128, (S,h)=8, b] -> [sbt, j, s2, S, h, b]
        arr = arr.reshape(8, 4, 32, 4, 2, 512)
        # -> [sbt, b, h, j, S, s2]; pos = 128*(4h+j) + 32S + s2
        arr = arr.transpose(0, 5, 4, 1, 3, 2)
        parts.append(arr.reshape(4096, 1024))
    out = np.concatenate(parts, axis=0)
    out += bias[None, :]
    return out


def kernel_v4(x, twiddle, bias, _repeats=1):
    xt, ca, hb, scale = _pack_v4(x, twiddle)
    nc = _build_v6(scale, repeats=_repeats)
    in_maps = [{"xt": xt[k], "ca": ca, "hb": hb} for k in range(N_CORES)]
    res = run_bass_kernel_spmd(nc, in_maps, list(range(N_CORES)))
    return _unpack_v4([r["out"] for r in res.results], scale, bias)



